# revision 1
# baseline (speedup 1.0000x reference)
"""Sharded causal attention kernel for trn2 (per-core program builder), v2.

Sharding: 8 cores = 2 batches x 4 head-groups (4 heads each).
v2 structure:
  - projections run on RAW x tiles as they stream in (rmsnorm scale is
    deferred: q/k scaled post-rotary along free dim, v scaled per-partition)
  - attention emits head-pair-adjacent matmuls (PE row/col tile packing)
  - output projection interleaved per q-block
"""

from contextlib import ExitStack

import numpy as np

import concourse.bass as bass
import concourse.mybir as mybir
import concourse.tile as tile
from concourse import bacc
from concourse.bass import _add_dep_helper as add_dep

f32 = mybir.dt.float32
f32r = mybir.dt.float32r
bf16 = mybir.dt.bfloat16
AF = mybir.ActivationFunctionType
OP = mybir.AluOpType

D = 1024
HPC = 4
DH = 64
ROT = 32
P = 128
EPS = 1e-8
NEG = -1e30


def build_program(n=2048, mm_dt="f32r", use_kmask=False, dbg=False):
    KT = D // P
    NQB = n // 512
    NTOK = n // P
    NCH = n // 512
    mdt = {"f32": f32, "f32r": f32r, "bf16": bf16}[mm_dt]
    nc = bacc.Bacc("TRN2", target_bir_lowering=False, debug=False)

    def din(name, shape, dt_):
        return nc.dram_tensor(name, shape, dt_, kind="ExternalInput")

    xT_d = din("xT", [D, n], mdt)
    wq_d = din("wq", [D, HPC * DH], mdt)
    wk_d = din("wk", [D, HPC * DH], mdt)
    wv_d = din("wv", [D, HPC * DH], mdt)
    wqr_d = din("wqr", [D, 2 * P], mdt)   # [h0r|0|h1r|0 , h2r|0|h3r|0]
    wkr_d = din("wkr", [D, 2 * P], mdt)
    wo_d = din("wo", [HPC * DH, D], mdt)
    cos_d = din("cos128", [P, n], f32)
    sin_d = din("sin128", [P, n], f32)
    tri_d = din("tri", [P, P], f32)
    id_d = din("ident", [P, P], f32)
    km_d = din("kmask", [P, NTOK], f32) if use_kmask else None
    out_d = nc.dram_tensor("out", [n, D], f32, kind="ExternalOutput")
    dbg_d = {}
    if dbg:
        for nm, shp in (("dqT0", [P, n]), ("dqT1", [P, n]), ("dkT0", [P, n]),
                        ("dv0", [P, HPC * (DH + 1)]), ("dden", [HPC, n]),
                        ("dattn0", [P, n]), ("drs", [1, n])):
            dbg_d[nm] = nc.dram_tensor(nm, shp, f32, kind="ExternalOutput")
        dbg_d["dpv"] = nc.dram_tensor("dpv", [P, 512], f32, kind="ExternalOutput")
        dbg_d["dbcd"] = nc.dram_tensor("dbcd", [64, 512], f32, kind="ExternalOutput")

    with tile.TileContext(nc) as tc, ExitStack() as top:
        persist = top.enter_context(tc.tile_pool(name="persist", bufs=1))
        ones_f32 = persist.tile([P, 1], f32, name="ones_f32")
        nc.vector.memset(ones_f32, 1.0)
        ones_col = persist.tile([P, 1], mdt, name="ones_col")
        nc.vector.tensor_copy(ones_col, ones_f32)
        ones_row = persist.tile([1, P], f32, name="ones_row")
        nc.vector.memset(ones_row, 1.0)
        tri_sb = persist.tile([P, P], f32, name="tri_sb")
        nc.sync.dma_start(out=tri_sb, in_=tri_d[:])
        ident_sb = persist.tile([P, P], f32, name="ident_sb")
        nc.sync.dma_start(out=ident_sb, in_=id_d[:])
        if use_kmask:
            km_sb = persist.tile([P, NTOK], f32, name="km_sb")
            nc.sync.dma_start(out=km_sb, in_=km_d[:])

        qkv = top.enter_context(tc.tile_pool(name="qkv", bufs=1))
        qT = [qkv.tile([P, n], mdt, name=f"qT{m}", tag=f"qT{m}") for m in range(2)]
        kT = [qkv.tile([P, n], mdt, name=f"kT{m}", tag=f"kT{m}") for m in range(2)]
        v_sb = [qkv.tile([P, HPC * (DH + 1)], mdt, name=f"v{tk}", tag=f"v{tk}")
                for tk in range(NTOK)]
        normk = top.enter_context(tc.tile_pool(name="normk", bufs=1))
        rs_col = normk.tile([P, NTOK], f32, name="rs_col")
        # per-q-block attention output chunks (freed after their out-proj)
        late = top.enter_context(tc.tile_pool(name="late", bufs=1))
        if dbg:
            den_sb = [normk.tile([1, n], f32, name=f"den{h}", tag=f"den{h}")
                      for h in range(HPC)]
        wop = top.enter_context(tc.tile_pool(name="wop", bufs=1))

        with ExitStack() as phase_a:
            big = phase_a.enter_context(tc.tile_pool(name="big", bufs=1))
            x_sb = [big.tile([P, n], mdt, name=f"x{t}", tag=f"x{t}") for t in range(KT)]
            for t in range(KT):
                nc.sync.dma_start(out=x_sb[t], in_=xT_d[t * P:(t + 1) * P, :])
            wq, wk, wv, wqr, wkr = [], [], [], [], []
            for t in range(KT):
                for lst, dsrc, w_, nm in (
                        (wq, wq_d, HPC * DH, "wq"), (wk, wk_d, HPC * DH, "wk"),
                        (wv, wv_d, HPC * DH, "wv"), (wqr, wqr_d, 2 * P, "wqr"),
                        (wkr, wkr_d, 2 * P, "wkr")):
                    tl = big.tile([P, w_], mdt, name=f"{nm}{t}", tag=f"{nm}{t}")
                    nc.sync.dma_start(out=tl, in_=dsrc[t * P:(t + 1) * P, :])
                    lst.append(tl)

            rot = phase_a.enter_context(tc.tile_pool(name="rot", bufs=1))
            cos_sb = rot.tile([P, n], f32, name="cos_sb")
            sin_sb = rot.tile([P, n], f32, name="sin_sb")

            # ---- rmsnorm scale (overlaps the projection matmuls below) ----
            last_rs_inst = None
            with tc.tile_pool(name="normt", bufs=1) as normt, \
                 tc.tile_pool(name="sqp", bufs=2) as sqp, \
                 tc.tile_pool(name="pnorm", bufs=1, space="PSUM") as pnorm, \
                 tc.tile_pool(name="pbc", bufs=2, space="PSUM") as pbc:
                ssq = [pnorm.tile([1, 512], f32, name=f"ssq{c}", tag=f"ssq{c}")
                       for c in range(NCH)]
                for t in range(KT):
                    for c in range(NCH):
                        sq = sqp.tile([P, 512], mdt, name=f"sq{t}_{c}", tag="sq")
                        nc.vector.tensor_mul(sq, x_sb[t][:, c * 512:(c + 1) * 512],
                                             x_sb[t][:, c * 512:(c + 1) * 512])
                        nc.tensor.matmul(ssq[c], ones_col, sq,
                                         start=(t == 0), stop=(t == KT - 1))
                s_row = normt.tile([1, n], f32, name="s_row")
                for c in range(NCH):
                    nc.scalar.activation(s_row[:, c * 512:(c + 1) * 512], ssq[c],
                                         AF.Sqrt, scale=1.0 / D)
                nc.vector.tensor_scalar_max(s_row, s_row, EPS)
                last_rs_inst = nc.vector.reciprocal(s_row, s_row)
                if dbg:
                    nc.sync.dma_start(out=dbg_d["drs"][:], in_=s_row)

                i1 = nc.sync.dma_start(out=cos_sb, in_=cos_d[:])
                i2 = nc.sync.dma_start(out=sin_sb, in_=sin_d[:])
                for i_ in (i1, i2):
                    add_dep(i_.ins, last_rs_inst.ins, True, "gate rot DMAs after norm")
                # fold rms scale into the rotary multipliers (reads bcast psum)
                for c in range(NCH):
                    bc = pbc.tile([P, 512], f32, name=f"bc{c}", tag="bc")
                    nc.tensor.matmul(bc, ones_row, s_row[:, c * 512:(c + 1) * 512],
                                     start=True, stop=True)
                    sl = slice(c * 512, (c + 1) * 512)
                    nc.vector.tensor_mul(cos_sb[:, sl], cos_sb[:, sl], bc)
                    nc.vector.tensor_mul(sin_sb[:, sl], sin_sb[:, sl], bc)
                    for tb in range(4):
                        tk = c * 4 + tb
                        dg = sqp.tile([P, P], f32, name=f"dg_{tk}", tag="dg")
                        nc.vector.tensor_mul(dg, bc[:, tb * P:(tb + 1) * P], ident_sb)
                        nc.vector.reduce_sum(rs_col[:, tk:tk + 1], dg,
                                             axis=mybir.AxisListType.X)

            with tc.tile_pool(name="pproj", bufs=3, space="PSUM") as pp, \
                 tc.tile_pool(name="ppv", bufs=3, space="PSUM") as ppv:
                rot_tail = []
                with tc.tile_pool(name="rotu", bufs=2) as rotu:
                    for base, wmain, wrot, nm_ in ((qT, wq, wqr, "q"), (kT, wk, wkr, "k")):
                        for c in range(NCH):
                            sl = slice(c * 512, (c + 1) * 512)
                            pss = []
                            for m in range(2):
                                ps = pp.tile([P, 512], f32,
                                             name=f"pp{nm_}{m}_{c}", tag="pp")
                                for t in range(KT):
                                    nc.tensor.matmul(
                                        ps, wmain[t][:, m * P:(m + 1) * P],
                                        x_sb[t][:, sl],
                                        start=(t == 0), stop=(t == KT - 1))
                                pss.append(ps)
                            for m in range(2):
                                nc.vector.tensor_mul(base[m][:, sl], pss[m],
                                                     cos_sb[:, sl])
                            for m in range(2):
                                psr = pp.tile([P, 512], f32,
                                              name=f"pp{nm_}r{m}_{c}", tag="pp")
                                for t in range(KT):
                                    nc.tensor.matmul(
                                        psr, wrot[t][:, m * P:(m + 1) * P],
                                        x_sb[t][:, sl],
                                        start=(t == 0), stop=(t == KT - 1))
                                u = rotu.tile([P, 512], f32,
                                              name=f"u_{nm_}{m}_{c}", tag="u")
                                nc.vector.tensor_mul(u, psr, sin_sb[:, sl])
                                rot_tail.append(
                                    nc.vector.tensor_add(base[m][:, sl],
                                                         base[m][:, sl], u))
                for tk in range(NTOK):
                    ps = ppv.tile([P, HPC * DH], f32, name=f"ppv_{tk}", tag="ppv")
                    for t in range(KT):
                        nc.tensor.matmul(ps, x_sb[t][:, tk * P:(tk + 1) * P], wv[t],
                                         start=(t == 0), stop=(t == KT - 1))
                    vv = v_sb[tk].rearrange("p (h c) -> p h c", h=HPC)
                    nc.vector.tensor_scalar_mul(
                        vv[:, :, 0:DH], ps.rearrange("p (h c) -> p h c", h=HPC),
                        rs_col[:, tk:tk + 1])
                    for hh in range(HPC):
                        nc.vector.tensor_copy(vv[:, hh, DH:DH + 1], ones_col)

        # wo loads once early-phase-A SBUF pressure has relaxed
        wo_sb = [wop.tile([P, D], mdt, name=f"wo{m}", tag=f"wo{m}") for m in range(2)]
        for m in range(2):
            iw = nc.sync.dma_start(out=wo_sb[m], in_=wo_d[m * P:(m + 1) * P, :])
            add_dep(iw.ins, rot_tail[-1].ins, True, "gate wo pool after rotary")

        # ---- attention + interleaved output projection ----
        with tc.tile_pool(name="ep", bufs=2) as ep, \
             tc.tile_pool(name="rbp", bufs=2) as rbp, \
             tc.tile_pool(name="bcdp", bufs=2) as bcdp, \
             tc.tile_pool(name="outsb", bufs=3) as osb, \
             tc.tile_pool(name="psim", bufs=1, space="PSUM") as psim, \
             tc.tile_pool(name="pmix", bufs=4, space="PSUM") as pmix:
            for qb in range(NQB):
                nkt = 4 * qb + 4
                qsl = slice(qb * 512, (qb + 1) * 512)
                attn = [late.tile([P, 512], mdt, name=f"attn{pr}_{qb}",
                                  tag=f"attn{pr}") for pr in range(2)]
                for pr in range(2):
                    pvh = [pmix.tile([DH + 1, 512], f32, name=f"pv_{pr}_{qb}_{h2}",
                                     tag="b512") for h2 in range(2)]
                    for g in range((nkt + 1) // 2):
                        kts = [z for z in (2 * g, 2 * g + 1) if z < nkt]
                        w_ = 512 * len(kts)
                        sims = [psim.tile([P, w_], f32, name=f"s{h2}_{pr}_{qb}_{g}",
                                          tag=f"sim{h2}") for h2 in range(2)]
                        for i, kt_ in enumerate(kts):
                            for h2 in range(2):
                                nc.tensor.matmul(
                                    sims[h2][:, i * 512:(i + 1) * 512],
                                    kT[pr][64 * h2:64 * h2 + 64, kt_ * P:(kt_ + 1) * P],
                                    qT[pr][64 * h2:64 * h2 + 64, qsl],
                                    start=True, stop=True, tile_position=(64 * h2, 0))
                        for i, kt_ in enumerate(kts):
                            d = kt_ - 4 * qb
                            for h2 in range(2):
                                if d >= 0:
                                    sl = sims[h2][:, i * 512 + d * P:i * 512 + (d + 1) * P]
                                    nc.vector.tensor_tensor(sl, sl, tri_sb, OP.add)
                                if use_kmask:
                                    sl = sims[h2][:, i * 512:(i + 1) * 512]
                                    nc.vector.tensor_scalar_add(sl, sl,
                                                                km_sb[:, kt_:kt_ + 1])
                        Es = [ep.tile([P, w_], mdt, name=f"E{h2}_{pr}_{qb}_{g}",
                                      tag=f"E{h2}") for h2 in range(2)]
                        for h2 in range(2):
                            nc.scalar.activation(Es[h2], sims[h2], AF.Exp)
                        for i, kt_ in enumerate(kts):
                            lo = max(0, kt_ - 4 * qb) * P
                            for h2 in range(2):
                                hh = 2 * pr + h2
                                nc.tensor.matmul(
                                    pvh[h2][:, lo:512],
                                    v_sb[kt_][:, (DH + 1) * hh:(DH + 1) * hh + DH + 1],
                                    Es[h2][:, i * 512 + lo:(i + 1) * 512],
                                    start=(kt_ == 0), stop=(kt_ == nkt - 1),
                                    skip_group_check=True)
                    for h2 in range(2):
                        if dbg:
                            nc.vector.tensor_copy(den_sb[2 * pr + h2][:, qsl],
                                                  pvh[h2][DH:DH + 1, :])
                        rb = rbp.tile([1, 512], f32, name=f"rb_{pr}_{qb}_{h2}", tag="rb")
                        nc.vector.reciprocal(rb, pvh[h2][DH:DH + 1, :])
                        bcd = bcdp.tile([DH, 512], f32, name=f"bcd_{pr}_{qb}_{h2}",
                                        tag="bcd")
                        bps = pmix.tile([DH, 512], f32, name=f"bps_{pr}_{qb}_{h2}",
                                        tag="b512")
                        nc.tensor.matmul(bps, ones_row[:, 0:DH], rb,
                                         start=True, stop=True)
                        nc.scalar.copy(bcd, bps)
                        nc.vector.tensor_tensor(
                            attn[pr][64 * h2:64 * h2 + 64, :], pvh[h2][0:DH, :],
                            bcd, OP.mult)
                # output projection for this q-block's token rows
                for tk in range(4 * qb, 4 * qb + 4):
                    tkl = tk - 4 * qb
                    for c2 in range(D // 512):
                        po = pmix.tile([P, 512], f32, name=f"po_{tk}_{c2}", tag="b512")
                        for m in range(2):
                            nc.tensor.matmul(po, attn[m][:, tkl * P:(tkl + 1) * P],
                                             wo_sb[m][:, c2 * 512:(c2 + 1) * 512],
                                             start=(m == 0), stop=(m == 1))
                        ob = osb.tile([P, 512], f32, name=f"ob_{tk}_{c2}", tag="ob")
                        nc.vector.tensor_copy(ob, po)
                        nc.sync.dma_start(
                            out=out_d[tk * P:(tk + 1) * P, c2 * 512:(c2 + 1) * 512],
                            in_=ob)
            if dbg:
                nc.sync.dma_start(out=dbg_d["dqT0"][:], in_=qT[0])
                nc.sync.dma_start(out=dbg_d["dqT1"][:], in_=qT[1])
                nc.sync.dma_start(out=dbg_d["dkT0"][:], in_=kT[0])
                nc.sync.dma_start(out=dbg_d["dv0"][:], in_=v_sb[0])

    nc.compile()
    return nc


# ---------------------------------------------------------------- host side

def np_dt(mm_dt):
    import ml_dtypes
    return {"f32": np.float32, "f32r": np.float32, "bf16": ml_dtypes.bfloat16}[mm_dt]


def make_core_inputs(x, mask, pos_emb, g, Wq, Wkv, Wo, core, n, mm_dt="f32r"):
    ndt = np_dt(mm_dt)
    b = core // 4
    h0 = (core % 4) * HPC
    scale = DH ** -0.5
    gW = Wq * g[:, None]
    gKV = Wkv * g[:, None]
    cols = slice(h0 * DH, (h0 + HPC) * DH)
    wq = gW[:, cols] * scale
    Wk_full = gKV[:, :D]
    Wv_full = gKV[:, D:]
    wk = Wk_full[:, cols]
    wv = Wv_full[:, cols]

    def rot_cols(W):
        # [h0r | 0 | h1r | 0, h2r | 0 | h3r | 0]: u tiles land aligned with qT
        out = np.zeros((D, 2 * P), dtype=W.dtype)
        for h in range(HPC):
            src = W[:, (h0 + h) * DH:(h0 + h) * DH + DH]
            base = h * DH
            out[:, base:base + 16] = -src[:, 16:32]
            out[:, base + 16:base + 32] = src[:, 0:16]
        return out

    wqr = rot_cols(gW) * scale
    wkr = rot_cols(Wk_full)
    wo = Wo[cols, :]

    cosf = np.cos(pos_emb.T).astype(np.float32)
    sinf = np.sin(pos_emb.T).astype(np.float32)
    cos128 = np.ones((P, n), np.float32)
    cos128[0:ROT] = cosf
    cos128[DH:DH + ROT] = cosf
    sin128 = np.zeros((P, n), np.float32)
    sin128[0:ROT] = sinf
    sin128[DH:DH + ROT] = sinf
    tri = np.where(np.arange(P)[:, None] <= np.arange(P)[None, :], 0.0, NEG
                   ).astype(np.float32)

    ins = {
        "xT": np.ascontiguousarray(x[b].T).astype(ndt),
        "wq": wq.astype(ndt), "wk": wk.astype(ndt), "wv": wv.astype(ndt),
        "wqr": wqr.astype(ndt), "wkr": wkr.astype(ndt), "wo": wo.astype(ndt),
        "cos128": cos128, "sin128": sin128, "tri": tri,
        "ident": np.eye(P, dtype=np.float32),
    }
    if not mask.all():
        km = np.where(mask[b], 0.0, NEG).astype(np.float32)
        ins["kmask"] = np.ascontiguousarray(km.reshape(n // P, P).T)
    return ins


# ---------------------------------------------------------------- runner

import os
import jax


def _run_per_device(nc, in_maps, core_ids):
    """Run the same Bass program independently on each visible device."""
    from concourse.bass2jax import (_bass_exec_p, install_neuronx_cc_hook,
                                    partition_id_tensor)
    install_neuronx_cc_hook()
    partition_name = nc.partition_id_tensor.name if nc.partition_id_tensor else None
    in_names, out_names, out_avals, zero_outs = [], [], [], []
    for alloc in nc.m.functions[0].allocations:
        if not isinstance(alloc, mybir.MemoryLocationSet):
            continue
        name = alloc.memorylocations[0].name
        if alloc.kind == "ExternalInput":
            if name != partition_name:
                in_names.append(name)
        elif alloc.kind == "ExternalOutput":
            out_names.append(name)
            shape = tuple(alloc.tensor_shape)
            dtype = mybir.dt.np(alloc.dtype)
            out_avals.append(jax.core.ShapedArray(shape, dtype))
            zero_outs.append(np.zeros(shape, dtype))
    n_params = len(in_names)
    all_in_names = list(in_names) + list(out_names)
    if partition_name is not None:
        all_in_names.append(partition_name)
    donate = tuple(range(n_params, n_params + len(out_names)))

    def _body(*args):
        operands = list(args)
        if partition_name is not None:
            operands.append(partition_id_tensor())
        outs = _bass_exec_p.bind(
            *operands, out_avals=tuple(out_avals), in_names=tuple(all_in_names),
            out_names=tuple(out_names), lowering_input_output_aliases=(),
            sim_require_finite=True, sim_require_nnan=True, nc=nc)
        return tuple(outs)

    fn = jax.jit(_body, donate_argnums=donate, keep_unused=True)
    futures = []
    for c, in_map in zip(core_ids, in_maps):
        dev = jax.devices()[c]
        args = [jax.device_put(np.asarray(in_map[nm]), dev) for nm in in_names]
        zz = [jax.device_put(z, dev) for z in zero_outs]
        futures.append(fn(*args, *zz))
    return [{nm: np.asarray(a) for nm, a in zip(out_names, f)} for f in futures]


_PROGRAM_CACHE = {}


def kernel(**inputs):
    os.environ.setdefault("NEURON_COMPILE_CACHE_URL", "/tmp/neuron_cache_kernel")
    x = np.asarray(inputs["x"], dtype=np.float32)
    mask = np.asarray(inputs["mask"]).astype(bool)
    pos_emb = np.asarray(inputs["pos_emb"], dtype=np.float32)
    g = np.asarray(inputs["g"], dtype=np.float32)
    Wq = np.asarray(inputs["Wq"], dtype=np.float32)
    Wkv = np.asarray(inputs["Wkv"], dtype=np.float32)
    Wo = np.asarray(inputs["Wo"], dtype=np.float32)
    bo = np.asarray(inputs["bo"], dtype=np.float32)
    b, n, _ = x.shape
    assert (b, n) == (2, 2048), (b, n)
    mm_dt = "f32r"
    use_km = not bool(mask.all())
    key = (n, mm_dt, use_km)
    if key not in _PROGRAM_CACHE:
        _PROGRAM_CACHE[key] = build_program(n=n, mm_dt=mm_dt, use_kmask=use_km)
    nc = _PROGRAM_CACHE[key]
    core_ids = list(range(8))
    in_maps = [make_core_inputs(x, mask, pos_emb, g, Wq, Wkv, Wo, c, n, mm_dt)
               for c in core_ids]
    results = _run_per_device(nc, in_maps, core_ids)
    out = np.zeros((b, n, D), np.float32)
    for c in core_ids:
        out[c // 4] += results[c]["out"]
    out += bo[None, None, :]
    return out



# revision 11
# speedup vs baseline: 1.3178x; 1.3178x over previous
"""Sharded causal attention kernel for trn2 (per-core program builder), v3.

Sharding: 8 cores = 2 batches x 4 head-groups (4 heads each).
v3 over v2:
  - causal mask applied POST-exp as a 0/1 multiply on Es (off the
    exp->exp critical chain; attention phase becomes Act-bound)
  - rot projections packed to [D,128] (4 heads x 32 rot dims) - halves
    their PE time; combined into qT/kT with 4 partition-offset adds
  - s_row/rb/ones_row are f32r so bc/bps matmuls run 1 cyc/row
  - attn normalization reads bps (PSUM) directly (no bcd stage on Act)
  - psum->sbuf out staging alternates DVE/Pool; bf16 IO halves DMA
"""

from contextlib import ExitStack

import numpy as np

import concourse.bass as bass
import concourse.mybir as mybir
import concourse.tile as tile
from concourse import bacc

f32 = mybir.dt.float32
f32r = mybir.dt.float32r
bf16 = mybir.dt.bfloat16
AF = mybir.ActivationFunctionType
OP = mybir.AluOpType

D = 1024
HPC = 4
DH = 64
ROT = 32
P = 128
EPS = 1e-8
NEG = -1e30


def build_program(n=2048, mm_dt="bf16", use_kmask=False):
    KT = D // P
    NQB = n // 512
    NTOK = n // P
    NCH = n // 512
    mdt = {"f32": f32, "f32r": f32r, "bf16": bf16}[mm_dt]
    nc = bacc.Bacc("TRN2", target_bir_lowering=False, debug=False)

    def din(name, shape, dt_):
        return nc.dram_tensor(name, shape, dt_, kind="ExternalInput")

    xT_d = din("xT", [D, n], mdt)
    wq_d = din("wq", [D, HPC * DH], mdt)
    wk_d = din("wk", [D, HPC * DH], mdt)
    wv_d = din("wv", [D, HPC * DH], mdt)
    wqr_d = din("wqr", [D, P], mdt)    # 4 heads x 32 rot cols
    wkr_d = din("wkr", [D, P], mdt)
    wo_d = din("wo", [HPC * DH, D], mdt)
    cos_d = din("cos128", [P, n], f32)  # rot rows cos, pass rows 1.0
    sin_d = din("sinc128", [P, n], f32)  # all four 32-row blocks = sin
    tri_d = din("tri01", [P, P], mdt)   # 1.0 where key<=query else 0.0
    id_d = din("ident", [P, P], f32)
    km_d = din("kmask", [P, NTOK], f32) if use_kmask else None
    out_d = nc.dram_tensor("out", [n, D], mdt, kind="ExternalOutput")

    with tile.TileContext(nc) as tc, ExitStack() as top:
        persist = top.enter_context(tc.tile_pool(name="persist", bufs=1))
        ones_f32 = persist.tile([P, 1], f32, name="ones_f32")
        nc.vector.memset(ones_f32, 1.0)
        ones_col = persist.tile([P, 1], mdt, name="ones_col")
        nc.vector.tensor_copy(ones_col, ones_f32)
        ones_row_f = persist.tile([1, P], f32, name="ones_row_f")
        nc.vector.memset(ones_row_f, 1.0)
        ones_row = persist.tile([1, P], f32r, name="ones_row")
        nc.vector.tensor_copy(ones_row, ones_row_f)
        tri_sb = persist.tile([P, P], mdt, name="tri_sb")
        nc.sync.dma_start(out=tri_sb, in_=tri_d[:])
        ident_sb = persist.tile([P, P], f32, name="ident_sb")
        nc.sync.dma_start(out=ident_sb, in_=id_d[:])
        if use_kmask:
            km_sb = persist.tile([P, NTOK], f32, name="km_sb")
            nc.sync.dma_start(out=km_sb, in_=km_d[:])

        qkv = top.enter_context(tc.tile_pool(name="qkv", bufs=1))
        qT = [qkv.tile([P, n], mdt, name=f"qT{m}", tag=f"qT{m}") for m in range(2)]
        kT = [qkv.tile([P, n], mdt, name=f"kT{m}", tag=f"kT{m}") for m in range(2)]
        v_sb = [qkv.tile([P, HPC * (DH + 1)], mdt, name=f"v{tk}", tag=f"v{tk}")
                for tk in range(NTOK)]
        normk = top.enter_context(tc.tile_pool(name="normk", bufs=1))
        rs_col = normk.tile([P, NTOK], f32, name="rs_col")
        # per-q-block attention output chunks (freed after their out-proj)
        late = top.enter_context(tc.tile_pool(name="late", bufs=1))
        wop = top.enter_context(tc.tile_pool(name="wop", bufs=1))

        with ExitStack() as phase_a:
            big = phase_a.enter_context(tc.tile_pool(name="big", bufs=1))
            # DMA issue order = consumption order: x, wq, wk, cos/sin,
            # wqr, wkr, wv, wo (the single DMA resource serializes them).
            x_sb = [big.tile([P, n], mdt, name=f"x{t}", tag=f"x{t}") for t in range(KT)]
            for t in range(KT):
                nc.sync.dma_start(out=x_sb[t], in_=xT_d[t * P:(t + 1) * P, :])

            def wload(dsrc, w_, nm):
                tls = []
                for t in range(KT):
                    tl = big.tile([P, w_], mdt, name=f"{nm}{t}", tag=f"{nm}{t}")
                    nc.sync.dma_start(out=tl, in_=dsrc[t * P:(t + 1) * P, :])
                    tls.append(tl)
                return tls

            wq = wload(wq_d, HPC * DH, "wq")
            wk = wload(wk_d, HPC * DH, "wk")
            rot = phase_a.enter_context(tc.tile_pool(name="rot", bufs=1))
            cos_sb = rot.tile([P, n], f32, name="cos_sb")
            sin_sb = rot.tile([P, n], f32, name="sin_sb")
            nc.sync.dma_start(out=cos_sb, in_=cos_d[:])
            nc.sync.dma_start(out=sin_sb, in_=sin_d[:])
            wqr = wload(wqr_d, P, "wqr")
            wkr = wload(wkr_d, P, "wkr")
            wv = wload(wv_d, HPC * DH, "wv")
            wo_sb = [wop.tile([P, D], mdt, name=f"wo{m}", tag=f"wo{m}")
                     for m in range(2)]
            for m in range(2):
                nc.sync.dma_start(out=wo_sb[m], in_=wo_d[m * P:(m + 1) * P, :])

            # ---- rmsnorm scale (overlaps the projection matmuls below) ----
            with tc.tile_pool(name="normt", bufs=1) as normt, \
                 tc.tile_pool(name="sqp", bufs=2) as sqp, \
                 tc.tile_pool(name="pnorm", bufs=1, space="PSUM") as pnorm, \
                 tc.tile_pool(name="pbc", bufs=2, space="PSUM") as pbc:
                ssq = [pnorm.tile([1, 512], f32, name=f"ssq{c}", tag=f"ssq{c}")
                       for c in range(NCH)]
                for t in range(KT):
                    for c in range(NCH):
                        sq = sqp.tile([P, 512], mdt, name=f"sq{t}_{c}", tag="sq")
                        nc.vector.tensor_mul(sq, x_sb[t][:, c * 512:(c + 1) * 512],
                                             x_sb[t][:, c * 512:(c + 1) * 512])
                        nc.tensor.matmul(ssq[c], ones_col, sq,
                                         start=(t == 0), stop=(t == KT - 1))
                s_row = normt.tile([1, n], f32r, name="s_row")
                for c in range(NCH):
                    nc.scalar.activation(s_row[:, c * 512:(c + 1) * 512], ssq[c],
                                         AF.Sqrt, scale=1.0 / D)
                with nc.allow_low_precision(reason="f32r has f32 bits"):
                    nc.vector.tensor_scalar_max(s_row, s_row, EPS)
                    nc.vector.reciprocal(s_row, s_row)

                # fold rms scale into the rotary multipliers (reads bcast psum)
                for c in range(NCH):
                    bc = pbc.tile([P, 512], f32, name=f"bc{c}", tag="bc")
                    nc.tensor.matmul(bc, ones_row, s_row[:, c * 512:(c + 1) * 512],
                                     start=True, stop=True)
                    sl = slice(c * 512, (c + 1) * 512)
                    nc.vector.tensor_mul(cos_sb[:, sl], cos_sb[:, sl], bc)
                    nc.vector.tensor_mul(sin_sb[:, sl], sin_sb[:, sl], bc)
                    for tb in range(4):
                        tk = c * 4 + tb
                        dg = sqp.tile([P, P], f32, name=f"dg_{tk}", tag="dg")
                        nc.vector.tensor_mul(dg, bc[:, tb * P:(tb + 1) * P], ident_sb)
                        nc.vector.reduce_sum(rs_col[:, tk:tk + 1], dg,
                                             axis=mybir.AxisListType.X)

            with tc.tile_pool(name="pproj", bufs=3, space="PSUM") as pp, \
                 tc.tile_pool(name="ppv", bufs=3, space="PSUM") as ppv:
                with tc.tile_pool(name="rotu", bufs=2, space="PSUM") as rotu:
                    for base, wmain, wrot, nm_ in ((qT, wq, wqr, "q"), (kT, wk, wkr, "k")):
                        for c in range(NCH):
                            sl = slice(c * 512, (c + 1) * 512)
                            pss = []
                            for m in range(2):
                                ps = pp.tile([P, 512], f32,
                                             name=f"pp{nm_}{m}_{c}", tag="pp")
                                for t in range(KT):
                                    nc.tensor.matmul(
                                        ps, wmain[t][:, m * P:(m + 1) * P],
                                        x_sb[t][:, sl],
                                        start=(t == 0), stop=(t == KT - 1))
                                pss.append(ps)
                            for m in range(2):
                                nc.vector.tensor_mul(base[m][:, sl], pss[m],
                                                     cos_sb[:, sl])
                            psr = pp.tile([P, 512], f32, name=f"pp{nm_}r_{c}", tag="pp")
                            for t in range(KT):
                                nc.tensor.matmul(psr, wrot[t], x_sb[t][:, sl],
                                                 start=(t == 0), stop=(t == KT - 1))
                            u = rotu.tile([P, 512], f32, name=f"u_{nm_}{c}", tag="u")
                            nc.vector.tensor_mul(u, psr, sin_sb[:, sl])
                            for h in range(HPC):
                                m, h2 = h // 2, h % 2
                                dst = base[m][64 * h2:64 * h2 + ROT, sl]
                                nc.vector.tensor_add(dst, dst,
                                                     u[ROT * h:ROT * (h + 1), :])
                for tk in range(NTOK):
                    ps = ppv.tile([P, HPC * DH], f32, name=f"ppv_{tk}", tag="ppv")
                    for t in range(KT):
                        nc.tensor.matmul(ps, x_sb[t][:, tk * P:(tk + 1) * P], wv[t],
                                         start=(t == 0), stop=(t == KT - 1))
                    vv = v_sb[tk].rearrange("p (h c) -> p h c", h=HPC)
                    nc.vector.tensor_scalar_mul(
                        vv[:, :, 0:DH], ps.rearrange("p (h c) -> p h c", h=HPC),
                        rs_col[:, tk:tk + 1])
                    for hh in range(HPC):
                        nc.vector.tensor_copy(vv[:, hh, DH:DH + 1], ones_col)

        # ---- attention + interleaved output projection ----
        with tc.tile_pool(name="ep", bufs=2) as ep, \
             tc.tile_pool(name="rbp", bufs=2) as rbp, \
             tc.tile_pool(name="bcdp", bufs=2) as bcdp, \
             tc.tile_pool(name="outsb", bufs=3) as osb, \
             tc.tile_pool(name="psim", bufs=1, space="PSUM") as psim, \
             tc.tile_pool(name="pmix", bufs=4, space="PSUM") as pmix:
            for qb in range(NQB):
                nkt = 4 * qb + 4
                qsl = slice(qb * 512, (qb + 1) * 512)
                attn = [late.tile([P, 512], mdt, name=f"attn{pr}_{qb}",
                                  tag=f"attn{pr}") for pr in range(2)]
                for pr in range(2):
                    pvh = [pmix.tile([DH + 1, 512], f32, name=f"pv_{pr}_{qb}_{h2}",
                                     tag="b512") for h2 in range(2)]
                    for g in range((nkt + 1) // 2):
                        kts = [z for z in (2 * g, 2 * g + 1) if z < nkt]
                        w_ = 512 * len(kts)
                        sims = [psim.tile([P, w_], f32, name=f"s{h2}_{pr}_{qb}_{g}",
                                          tag=f"sim{h2}") for h2 in range(2)]
                        for i, kt_ in enumerate(kts):
                            for h2 in range(2):
                                nc.tensor.matmul(
                                    sims[h2][:, i * 512:(i + 1) * 512],
                                    kT[pr][64 * h2:64 * h2 + 64, kt_ * P:(kt_ + 1) * P],
                                    qT[pr][64 * h2:64 * h2 + 64, qsl],
                                    start=True, stop=True, tile_position=(64 * h2, 0))
                        if use_kmask:
                            for i, kt_ in enumerate(kts):
                                for h2 in range(2):
                                    sl = sims[h2][:, i * 512:(i + 1) * 512]
                                    nc.vector.tensor_scalar_add(sl, sl,
                                                                km_sb[:, kt_:kt_ + 1])
                        Es = [ep.tile([P, w_], mdt, name=f"E{h2}_{pr}_{qb}_{g}",
                                      tag=f"E{h2}") for h2 in range(2)]
                        for h2 in range(2):
                            nc.scalar.activation(Es[h2], sims[h2], AF.Exp)
                        # causal mask: zero the partial-triangle block post-exp
                        for i, kt_ in enumerate(kts):
                            d = kt_ - 4 * qb
                            if d >= 0:
                                for h2 in range(2):
                                    sl = Es[h2][:, i * 512 + d * P:i * 512 + (d + 1) * P]
                                    nc.vector.tensor_mul(sl, sl, tri_sb)
                        for i, kt_ in enumerate(kts):
                            lo = max(0, kt_ - 4 * qb) * P
                            for h2 in range(2):
                                hh = 2 * pr + h2
                                nc.tensor.matmul(
                                    pvh[h2][:, lo:512],
                                    v_sb[kt_][:, (DH + 1) * hh:(DH + 1) * hh + DH + 1],
                                    Es[h2][:, i * 512 + lo:(i + 1) * 512],
                                    start=(kt_ == 0), stop=(kt_ == nkt - 1),
                                    skip_group_check=True)
                    for h2 in range(2):
                        rb = rbp.tile([1, 512], f32r, name=f"rb_{pr}_{qb}_{h2}",
                                      tag="rb")
                        with nc.allow_low_precision(reason="f32r has f32 bits"):
                            nc.vector.reciprocal(rb, pvh[h2][DH:DH + 1, :])
                        bps = pmix.tile([DH, 512], f32, name=f"bps_{pr}_{qb}_{h2}",
                                        tag="b512")
                        nc.tensor.matmul(bps, ones_row[:, 0:DH], rb,
                                         start=True, stop=True)
                        bcd = bcdp.tile([DH, 512], f32, name=f"bcd_{pr}_{qb}_{h2}",
                                        tag="bcd")
                        nc.vector.tensor_copy(bcd, bps)
                        nc.vector.tensor_tensor(
                            attn[pr][64 * h2:64 * h2 + 64, :], pvh[h2][0:DH, :],
                            bcd, OP.mult)
                # output projection for this q-block's token rows
                for tk in range(4 * qb, 4 * qb + 4):
                    tkl = tk - 4 * qb
                    for c2 in range(D // 512):
                        po = pmix.tile([P, 512], f32, name=f"po_{tk}_{c2}", tag="b512")
                        for m in range(2):
                            nc.tensor.matmul(po, attn[m][:, tkl * P:(tkl + 1) * P],
                                             wo_sb[m][:, c2 * 512:(c2 + 1) * 512],
                                             start=(m == 0), stop=(m == 1))
                        ob = osb.tile([P, 512], mdt, name=f"ob_{tk}_{c2}", tag="ob")
                        nc.vector.tensor_copy(ob, po)
                        nc.sync.dma_start(
                            out=out_d[tk * P:(tk + 1) * P, c2 * 512:(c2 + 1) * 512],
                            in_=ob)

    nc.compile()
    return nc


# ---------------------------------------------------------------- host side

def np_dt(mm_dt):
    import ml_dtypes
    return {"f32": np.float32, "f32r": np.float32, "bf16": ml_dtypes.bfloat16}[mm_dt]


def make_core_inputs(x, mask, pos_emb, g, Wq, Wkv, Wo, core, n, mm_dt="bf16"):
    ndt = np_dt(mm_dt)
    b = core // 4
    h0 = (core % 4) * HPC
    scale = DH ** -0.5
    gW = Wq * g[:, None]
    gKV = Wkv * g[:, None]
    cols = slice(h0 * DH, (h0 + HPC) * DH)
    wq = gW[:, cols] * scale
    Wk_full = gKV[:, :D]
    Wv_full = gKV[:, D:]
    wk = Wk_full[:, cols]
    wv = Wv_full[:, cols]

    def rot_cols(W):
        # [h0:32 | h1:32 | h2:32 | h3:32] rotate-half columns
        out = np.zeros((D, P), dtype=W.dtype)
        for h in range(HPC):
            src = W[:, (h0 + h) * DH:(h0 + h) * DH + DH]
            base = h * ROT
            out[:, base:base + 16] = -src[:, 16:32]
            out[:, base + 16:base + 32] = src[:, 0:16]
        return out

    wqr = rot_cols(gW) * scale
    wkr = rot_cols(Wk_full)
    wo = Wo[cols, :]

    cosf = np.cos(pos_emb.T).astype(np.float32)
    sinf = np.sin(pos_emb.T).astype(np.float32)
    cos128 = np.ones((P, n), np.float32)
    cos128[0:ROT] = cosf
    cos128[DH:DH + ROT] = cosf
    sinc128 = np.empty((P, n), np.float32)
    for h in range(HPC):
        sinc128[h * ROT:(h + 1) * ROT] = sinf
    tri01 = (np.arange(P)[:, None] <= np.arange(P)[None, :]).astype(np.float32)

    ins = {
        "xT": np.ascontiguousarray(x[b].T).astype(ndt),
        "wq": wq.astype(ndt), "wk": wk.astype(ndt), "wv": wv.astype(ndt),
        "wqr": wqr.astype(ndt), "wkr": wkr.astype(ndt), "wo": wo.astype(ndt),
        "cos128": cos128, "sinc128": sinc128, "tri01": tri01.astype(ndt),
        "ident": np.eye(P, dtype=np.float32),
    }
    if not mask.all():
        km = np.where(mask[b], 0.0, NEG).astype(np.float32)
        ins["kmask"] = np.ascontiguousarray(km.reshape(n // P, P).T)
    return ins


# ---------------------------------------------------------------- runner

import os
import jax


def _run_per_device(nc, in_maps, core_ids):
    """Run the same Bass program independently on each visible device."""
    from concourse.bass2jax import (_bass_exec_p, install_neuronx_cc_hook,
                                    partition_id_tensor)
    install_neuronx_cc_hook()
    partition_name = nc.partition_id_tensor.name if nc.partition_id_tensor else None
    in_names, out_names, out_avals, zero_outs = [], [], [], []
    for alloc in nc.m.functions[0].allocations:
        if not isinstance(alloc, mybir.MemoryLocationSet):
            continue
        name = alloc.memorylocations[0].name
        if alloc.kind == "ExternalInput":
            if name != partition_name:
                in_names.append(name)
        elif alloc.kind == "ExternalOutput":
            out_names.append(name)
            shape = tuple(alloc.tensor_shape)
            dtype = mybir.dt.np(alloc.dtype)
            out_avals.append(jax.core.ShapedArray(shape, dtype))
            zero_outs.append(np.zeros(shape, dtype))
    n_params = len(in_names)
    all_in_names = list(in_names) + list(out_names)
    if partition_name is not None:
        all_in_names.append(partition_name)
    donate = tuple(range(n_params, n_params + len(out_names)))

    def _body(*args):
        operands = list(args)
        if partition_name is not None:
            operands.append(partition_id_tensor())
        outs = _bass_exec_p.bind(
            *operands, out_avals=tuple(out_avals), in_names=tuple(all_in_names),
            out_names=tuple(out_names), lowering_input_output_aliases=(),
            sim_require_finite=True, sim_require_nnan=True, nc=nc)
        return tuple(outs)

    fn = jax.jit(_body, donate_argnums=donate, keep_unused=True)
    futures = []
    for c, in_map in zip(core_ids, in_maps):
        dev = jax.devices()[c]
        args = [jax.device_put(np.asarray(in_map[nm]), dev) for nm in in_names]
        zz = [jax.device_put(z, dev) for z in zero_outs]
        futures.append(fn(*args, *zz))
    return [{nm: np.asarray(a) for nm, a in zip(out_names, f)} for f in futures]


_PROGRAM_CACHE = {}

MM_DT = "bf16"


def kernel(**inputs):
    os.environ.setdefault("NEURON_COMPILE_CACHE_URL", "/tmp/neuron_cache_kernel")
    x = np.asarray(inputs["x"], dtype=np.float32)
    mask = np.asarray(inputs["mask"]).astype(bool)
    pos_emb = np.asarray(inputs["pos_emb"], dtype=np.float32)
    g = np.asarray(inputs["g"], dtype=np.float32)
    Wq = np.asarray(inputs["Wq"], dtype=np.float32)
    Wkv = np.asarray(inputs["Wkv"], dtype=np.float32)
    Wo = np.asarray(inputs["Wo"], dtype=np.float32)
    bo = np.asarray(inputs["bo"], dtype=np.float32)
    b, n, _ = x.shape
    assert (b, n) == (2, 2048), (b, n)
    mm_dt = MM_DT
    use_km = not bool(mask.all())
    key = (n, mm_dt, use_km)
    if key not in _PROGRAM_CACHE:
        _PROGRAM_CACHE[key] = build_program(n=n, mm_dt=mm_dt, use_kmask=use_km)
    nc = _PROGRAM_CACHE[key]
    core_ids = list(range(8))
    in_maps = [make_core_inputs(x, mask, pos_emb, g, Wq, Wkv, Wo, c, n, mm_dt)
               for c in core_ids]
    results = _run_per_device(nc, in_maps, core_ids)
    out = np.zeros((b, n, D), np.float32)
    for c in core_ids:
        out[c // 4] += results[c]["out"].astype(np.float32)
    out += bo[None, None, :]
    return out


# revision 14
# speedup vs baseline: 1.4487x; 1.0994x over previous
"""Sharded causal attention kernel for trn2 (per-core program builder), v4.

Sharding: 8 cores = 2 batches x 4 head-groups (4 heads each).
v4 over v3:
  - attention g-loop software-pipelined: sims(g+1) issue before pv(g)
    so the Act engine runs exps back-to-back (Act-bound inner loop)
  - rmsnorm: squares split DVE/Act, abs_rsqrt on Act replaces
    sqrt+max+reciprocal; ssq packed in one PSUM bank ([4,512])
  - rotary: psr scaled in-place in PSUM, staged to SBUF by Act copy,
    merged into qT/kT via gpsimd DMA accumulate-adds (DVE freed)
  - weights loaded as one DMA per tensor (HWDGE is ~625ns/instr)
  - output staged per token-tile ([128,1024]) halving store DMAs
"""

from contextlib import ExitStack

import numpy as np

import concourse.bass as bass
import concourse.mybir as mybir
import concourse.tile as tile
from concourse import bacc

f32 = mybir.dt.float32
f32r = mybir.dt.float32r
bf16 = mybir.dt.bfloat16
AF = mybir.ActivationFunctionType
OP = mybir.AluOpType

D = 1024
HPC = 4
DH = 64
ROT = 32
P = 128
NEG = -1e30


def build_program(n=2048, mm_dt="bf16", use_kmask=False):
    KT = D // P
    NQB = n // 512
    NTOK = n // P
    NCH = n // 512
    mdt = {"f32": f32, "f32r": f32r, "bf16": bf16}[mm_dt]
    nc = bacc.Bacc("TRN2", target_bir_lowering=False, debug=False)

    def din(name, shape, dt_):
        return nc.dram_tensor(name, shape, dt_, kind="ExternalInput")

    xT_d = din("xT", [D, n], mdt)
    # weights come in t-major packed layout [128, KT*cols] (one DMA each)
    wq_d = din("wq", [P, KT * HPC * DH], mdt)
    wk_d = din("wk", [P, KT * HPC * DH], mdt)
    wv_d = din("wv", [P, KT * HPC * DH], mdt)
    wqr_d = din("wqr", [P, KT * P], mdt)   # 4 heads x 32 rot cols per t
    wkr_d = din("wkr", [P, KT * P], mdt)
    wo_d = din("wo", [P, 2 * D], mdt)
    cos_d = din("cos128", [P, n], f32)   # rot rows cos, pass rows 1.0
    sin_d = din("sinc128", [P, n], f32)  # all four 32-row blocks = sin
    tri_d = din("tri01", [P, P], mdt)    # 1.0 where key<=query else 0.0
    id_d = din("ident", [P, P], f32)
    km_d = din("kmask", [P, NTOK], f32) if use_kmask else None
    out_d = nc.dram_tensor("out", [n, D], mdt, kind="ExternalOutput")

    with tile.TileContext(nc) as tc, ExitStack() as top:
        persist = top.enter_context(tc.tile_pool(name="persist", bufs=1))
        ones_f32 = persist.tile([P, 1], f32, name="ones_f32")
        nc.vector.memset(ones_f32, 1.0)
        ones_col = persist.tile([P, 1], mdt, name="ones_col")
        nc.vector.tensor_copy(ones_col, ones_f32)
        ones_row_f = persist.tile([1, P], f32, name="ones_row_f")
        nc.vector.memset(ones_row_f, 1.0)
        ones_row = persist.tile([1, P], f32r, name="ones_row")
        nc.vector.tensor_copy(ones_row, ones_row_f)

        qkv = top.enter_context(tc.tile_pool(name="qkv", bufs=1))
        qT = [qkv.tile([P, n], mdt, name=f"qT{m}", tag=f"qT{m}") for m in range(2)]
        kT = [qkv.tile([P, n], mdt, name=f"kT{m}", tag=f"kT{m}") for m in range(2)]
        v_sb = [qkv.tile([P, HPC * (DH + 1)], mdt, name=f"v{tk}", tag=f"v{tk}")
                for tk in range(NTOK)]
        normk = top.enter_context(tc.tile_pool(name="normk", bufs=1))
        rs_col = normk.tile([P, NTOK], f32, name="rs_col")
        late = top.enter_context(tc.tile_pool(name="late", bufs=1))
        wop = top.enter_context(tc.tile_pool(name="wop", bufs=1))

        with ExitStack() as phase_a:
            big = phase_a.enter_context(tc.tile_pool(name="big", bufs=1))
            # DMA issue order = consumption order (single serialized DMA).
            x_sb = [big.tile([P, n], mdt, name=f"x{t}", tag=f"x{t}") for t in range(KT)]
            for t in range(KT):
                nc.sync.dma_start(out=x_sb[t], in_=xT_d[t * P:(t + 1) * P, :])

            def wload(dsrc, w_, nm):
                tl = big.tile([P, KT * w_], mdt, name=nm, tag=nm)
                nc.sync.dma_start(out=tl, in_=dsrc[:])
                return [tl[:, t * w_:(t + 1) * w_] for t in range(KT)]

            wq = wload(wq_d, HPC * DH, "wq")
            wk = wload(wk_d, HPC * DH, "wk")
            rot = phase_a.enter_context(tc.tile_pool(name="rot", bufs=1))
            cos_sb = rot.tile([P, n], f32, name="cos_sb")
            sin_sb = rot.tile([P, n], f32, name="sin_sb")
            nc.sync.dma_start(out=cos_sb, in_=cos_d[:])
            nc.sync.dma_start(out=sin_sb, in_=sin_d[:])
            wqr = wload(wqr_d, P, "wqr")
            wkr = wload(wkr_d, P, "wkr")
            wv = wload(wv_d, HPC * DH, "wv")
            wo_all = wop.tile([P, 2 * D], mdt, name="wo_all")
            nc.sync.dma_start(out=wo_all, in_=wo_d[:])
            wo_sb = [wo_all[:, m * D:(m + 1) * D] for m in range(2)]
            ident_sb = persist.tile([P, P], f32, name="ident_sb")
            nc.sync.dma_start(out=ident_sb, in_=id_d[:])
            tri_sb = persist.tile([P, P], mdt, name="tri_sb")
            nc.sync.dma_start(out=tri_sb, in_=tri_d[:])
            if use_kmask:
                km_sb = persist.tile([P, NTOK], f32, name="km_sb")
                nc.sync.dma_start(out=km_sb, in_=km_d[:])

            with tc.tile_pool(name="pproj", bufs=3, space="PSUM") as pp, \
                 tc.tile_pool(name="pnorm", bufs=1, space="PSUM") as pnorm, \
                 tc.tile_pool(name="pbc", bufs=1, space="PSUM") as pbc, \
                 tc.tile_pool(name="ppv", bufs=2, space="PSUM") as ppv, \
                 tc.tile_pool(name="normt", bufs=1) as normt, \
                 tc.tile_pool(name="sqp", bufs=2) as sqp, \
                 tc.tile_pool(name="usp", bufs=2) as usp:
                # ---- rmsnorm (overlaps x streaming; sq split DVE/Act) ----
                # matmul PSUM writes must start at partition 0/32/64: two
                # chunks per bank at partitions {0, 64}
                ssq2 = [pnorm.tile([P, 512], f32, name=f"ssq2_{i}", tag=f"ssq2_{i}")
                        for i in range(2)]
                sloc = [(ssq2[c // 2], 64 * (c % 2)) for c in range(NCH)]
                for t in range(KT):
                    for c in range(NCH):
                        sq = sqp.tile([P, 512], mdt, name=f"sq{t}_{c}", tag="sq")
                        xs = x_sb[t][:, c * 512:(c + 1) * 512]
                        if (t * NCH + c) % 2 == 0:
                            nc.vector.tensor_mul(sq, xs, xs)
                        else:
                            nc.scalar.activation(sq, xs, AF.Square)
                        stile, soff = sloc[c]
                        nc.tensor.matmul(stile[soff:soff + 1, :], ones_col, sq,
                                         start=(t == 0), stop=(t == KT - 1))
                s_row = normt.tile([1, n], f32r, name="s_row")
                for c in range(NCH):
                    sl = slice(c * 512, (c + 1) * 512)
                    stile, soff = sloc[c]
                    # s = 1/sqrt(ssq/D); matches 1/max(sqrt(.), eps) for all
                    # realistic (nonzero) token rows
                    with nc.allow_low_precision(reason="f32r has f32 bits"):
                        nc.scalar.activation(s_row[:, sl], stile[soff:soff + 1, :],
                                             AF.Abs_reciprocal_sqrt, scale=1.0 / D)
                    bc = pbc.tile([P, 512], f32, name=f"bc{c}", tag="bc")
                    nc.tensor.matmul(bc, ones_row, s_row[:, sl],
                                     start=True, stop=True)
                    nc.vector.tensor_mul(cos_sb[:, sl], cos_sb[:, sl], bc)
                    nc.vector.tensor_mul(sin_sb[:, sl], sin_sb[:, sl], bc)
                    for tb in range(4):
                        tk = c * 4 + tb
                        dg = sqp.tile([P, P], f32, name=f"dg_{tk}", tag="dg")
                        nc.vector.tensor_mul(dg, bc[:, tb * P:(tb + 1) * P], ident_sb)
                        nc.vector.reduce_sum(rs_col[:, tk:tk + 1], dg,
                                             axis=mybir.AxisListType.X)

                # ---- q/k projections with fused rotary ----
                for base, wmain, wrot, nm_ in ((qT, wq, wqr, "q"), (kT, wk, wkr, "k")):
                    for c in range(NCH):
                        sl = slice(c * 512, (c + 1) * 512)
                        pss = []
                        for m in range(2):
                            ps = pp.tile([P, 512], f32,
                                         name=f"pp{nm_}{m}_{c}", tag="pp")
                            for t in range(KT):
                                nc.tensor.matmul(
                                    ps, wmain[t][:, m * P:(m + 1) * P],
                                    x_sb[t][:, sl],
                                    start=(t == 0), stop=(t == KT - 1))
                            pss.append(ps)
                        for m in range(2):
                            nc.vector.tensor_mul(base[m][:, sl], pss[m],
                                                 cos_sb[:, sl])
                        psr = pp.tile([P, 512], f32, name=f"pp{nm_}r_{c}", tag="pp")
                        for t in range(KT):
                            nc.tensor.matmul(psr, wrot[t], x_sb[t][:, sl],
                                             start=(t == 0), stop=(t == KT - 1))
                        nc.vector.tensor_mul(psr, psr, sin_sb[:, sl])
                        u = usp.tile([P, 512], mdt, name=f"u_{nm_}{c}", tag="u")
                        nc.scalar.copy(u, psr)
                        for h in range(HPC):
                            m, h2 = h // 2, h % 2
                            nc.gpsimd.dma_start(
                                out=base[m][64 * h2:64 * h2 + ROT, sl],
                                in_=u[ROT * h:ROT * (h + 1), :],
                                accum_op=OP.add)

                # ---- v projection ----
                for tk in range(NTOK):
                    ps = ppv.tile([P, HPC * DH], f32, name=f"ppv_{tk}", tag="ppv")
                    for t in range(KT):
                        nc.tensor.matmul(ps, x_sb[t][:, tk * P:(tk + 1) * P], wv[t],
                                         start=(t == 0), stop=(t == KT - 1))
                    vv = v_sb[tk].rearrange("p (h c) -> p h c", h=HPC)
                    nc.vector.tensor_scalar_mul(
                        vv[:, :, 0:DH], ps.rearrange("p (h c) -> p h c", h=HPC),
                        rs_col[:, tk:tk + 1])
                    for hh in range(HPC):
                        nc.vector.tensor_copy(vv[:, hh, DH:DH + 1], ones_col)

        # ---- attention (software-pipelined) + output projection ----
        with tc.tile_pool(name="ep", bufs=2) as ep, \
             tc.tile_pool(name="rbp", bufs=2) as rbp, \
             tc.tile_pool(name="bcdp", bufs=2) as bcdp, \
             tc.tile_pool(name="outsb", bufs=3) as osb, \
             tc.tile_pool(name="psim", bufs=1, space="PSUM") as psim, \
             tc.tile_pool(name="pmix", bufs=4, space="PSUM") as pmix:
            for qb in range(NQB):
                nkt = 4 * qb + 4
                ng = (nkt + 1) // 2
                qsl = slice(qb * 512, (qb + 1) * 512)
                attn = [late.tile([P, 512], mdt, name=f"attn{pr}_{qb}",
                                  tag=f"attn{pr}") for pr in range(2)]
                for pr in range(2):
                    pvh = [pmix.tile([DH + 1, 512], f32, name=f"pv_{pr}_{qb}_{h2}",
                                     tag="b512") for h2 in range(2)]

                    def emit_sims(g):
                        kts = [z for z in (2 * g, 2 * g + 1) if z < nkt]
                        w_ = 512 * len(kts)
                        sims = [psim.tile([P, w_], f32, name=f"s{h2}_{pr}_{qb}_{g}",
                                          tag=f"sim{h2}") for h2 in range(2)]
                        for i, kt_ in enumerate(kts):
                            for h2 in range(2):
                                nc.tensor.matmul(
                                    sims[h2][:, i * 512:(i + 1) * 512],
                                    kT[pr][64 * h2:64 * h2 + 64,
                                           kt_ * P:(kt_ + 1) * P],
                                    qT[pr][64 * h2:64 * h2 + 64, qsl],
                                    start=True, stop=True,
                                    tile_position=(64 * h2, 0))
                        if use_kmask:
                            for i, kt_ in enumerate(kts):
                                for h2 in range(2):
                                    sl = sims[h2][:, i * 512:(i + 1) * 512]
                                    nc.vector.tensor_scalar_add(
                                        sl, sl, km_sb[:, kt_:kt_ + 1])
                        return sims, kts

                    cur = emit_sims(0)
                    for g in range(ng):
                        sims, kts = cur
                        w_ = 512 * len(kts)
                        Es = [ep.tile([P, w_], mdt, name=f"E{h2}_{pr}_{qb}_{g}",
                                      tag=f"E{h2}") for h2 in range(2)]
                        for h2 in range(2):
                            nc.scalar.activation(Es[h2], sims[h2], AF.Exp)
                        for i, kt_ in enumerate(kts):
                            d = kt_ - 4 * qb
                            if d >= 0:
                                for h2 in range(2):
                                    sl = Es[h2][:, i * 512 + d * P:
                                                i * 512 + (d + 1) * P]
                                    nc.vector.tensor_mul(sl, sl, tri_sb)
                        if g + 1 < ng:
                            cur = emit_sims(g + 1)
                        for i, kt_ in enumerate(kts):
                            lo = max(0, kt_ - 4 * qb) * P
                            for h2 in range(2):
                                hh = 2 * pr + h2
                                nc.tensor.matmul(
                                    pvh[h2][:, lo:512],
                                    v_sb[kt_][:, (DH + 1) * hh:
                                              (DH + 1) * hh + DH + 1],
                                    Es[h2][:, i * 512 + lo:(i + 1) * 512],
                                    start=(kt_ == 0), stop=(kt_ == nkt - 1),
                                    skip_group_check=True)
                    for h2 in range(2):
                        rb = rbp.tile([1, 512], f32r, name=f"rb_{pr}_{qb}_{h2}",
                                      tag="rb")
                        with nc.allow_low_precision(reason="f32r has f32 bits"):
                            nc.vector.reciprocal(rb, pvh[h2][DH:DH + 1, :])
                        bps = pmix.tile([DH, 512], f32, name=f"bps_{pr}_{qb}_{h2}",
                                        tag="b512")
                        nc.tensor.matmul(bps, ones_row[:, 0:DH], rb,
                                         start=True, stop=True)
                        bcd = bcdp.tile([DH, 512], f32, name=f"bcd_{pr}_{qb}_{h2}",
                                        tag="bcd")
                        nc.vector.tensor_copy(bcd, bps)
                        nc.vector.tensor_tensor(
                            attn[pr][64 * h2:64 * h2 + 64, :], pvh[h2][0:DH, :],
                            bcd, OP.mult)
                # output projection for this q-block's token rows
                for tk in range(4 * qb, 4 * qb + 4):
                    tkl = tk - 4 * qb
                    ob = osb.tile([P, D], mdt, name=f"ob_{tk}", tag="ob")
                    for c2 in range(D // 512):
                        po = pmix.tile([P, 512], f32, name=f"po_{tk}_{c2}",
                                       tag="b512")
                        for m in range(2):
                            nc.tensor.matmul(po, attn[m][:, tkl * P:(tkl + 1) * P],
                                             wo_sb[m][:, c2 * 512:(c2 + 1) * 512],
                                             start=(m == 0), stop=(m == 1))
                        nc.vector.tensor_copy(ob[:, c2 * 512:(c2 + 1) * 512], po)
                    nc.sync.dma_start(out=out_d[tk * P:(tk + 1) * P, :], in_=ob)

    nc.compile()
    return nc


# ---------------------------------------------------------------- host side

def np_dt(mm_dt):
    import ml_dtypes
    return {"f32": np.float32, "f32r": np.float32, "bf16": ml_dtypes.bfloat16}[mm_dt]


def _tmajor(W):
    """[D, cols] -> [128, KT*cols] t-major packing for single-DMA load."""
    KT = W.shape[0] // P
    return np.concatenate([W[t * P:(t + 1) * P, :] for t in range(KT)], axis=1)


def make_core_inputs(x, mask, pos_emb, g, Wq, Wkv, Wo, core, n, mm_dt="bf16"):
    ndt = np_dt(mm_dt)
    b = core // 4
    h0 = (core % 4) * HPC
    scale = DH ** -0.5
    gW = Wq * g[:, None]
    gKV = Wkv * g[:, None]
    cols = slice(h0 * DH, (h0 + HPC) * DH)
    wq = gW[:, cols] * scale
    Wk_full = gKV[:, :D]
    Wv_full = gKV[:, D:]
    wk = Wk_full[:, cols]
    wv = Wv_full[:, cols]

    def rot_cols(W):
        # [h0:32 | h1:32 | h2:32 | h3:32] rotate-half columns
        out = np.zeros((D, P), dtype=W.dtype)
        for h in range(HPC):
            src = W[:, (h0 + h) * DH:(h0 + h) * DH + DH]
            base = h * ROT
            out[:, base:base + 16] = -src[:, 16:32]
            out[:, base + 16:base + 32] = src[:, 0:16]
        return out

    wqr = rot_cols(gW) * scale
    wkr = rot_cols(Wk_full)
    wo = np.concatenate([Wo[cols, :][m * P:(m + 1) * P, :] for m in range(2)],
                        axis=1)

    cosf = np.cos(pos_emb.T).astype(np.float32)
    sinf = np.sin(pos_emb.T).astype(np.float32)
    cos128 = np.ones((P, n), np.float32)
    cos128[0:ROT] = cosf
    cos128[DH:DH + ROT] = cosf
    sinc128 = np.empty((P, n), np.float32)
    for h in range(HPC):
        sinc128[h * ROT:(h + 1) * ROT] = sinf
    tri01 = (np.arange(P)[:, None] <= np.arange(P)[None, :]).astype(np.float32)

    ins = {
        "xT": np.ascontiguousarray(x[b].T).astype(ndt),
        "wq": _tmajor(wq).astype(ndt), "wk": _tmajor(wk).astype(ndt),
        "wv": _tmajor(wv).astype(ndt), "wqr": _tmajor(wqr).astype(ndt),
        "wkr": _tmajor(wkr).astype(ndt), "wo": wo.astype(ndt),
        "cos128": cos128, "sinc128": sinc128, "tri01": tri01.astype(ndt),
        "ident": np.eye(P, dtype=np.float32),
    }
    if not mask.all():
        km = np.where(mask[b], 0.0, NEG).astype(np.float32)
        ins["kmask"] = np.ascontiguousarray(km.reshape(n // P, P).T)
    return ins


# ---------------------------------------------------------------- runner

import os
import jax


def _run_per_device(nc, in_maps, core_ids):
    """Run the same Bass program independently on each visible device."""
    from concourse.bass2jax import (_bass_exec_p, install_neuronx_cc_hook,
                                    partition_id_tensor)
    install_neuronx_cc_hook()
    partition_name = nc.partition_id_tensor.name if nc.partition_id_tensor else None
    in_names, out_names, out_avals, zero_outs = [], [], [], []
    for alloc in nc.m.functions[0].allocations:
        if not isinstance(alloc, mybir.MemoryLocationSet):
            continue
        name = alloc.memorylocations[0].name
        if alloc.kind == "ExternalInput":
            if name != partition_name:
                in_names.append(name)
        elif alloc.kind == "ExternalOutput":
            out_names.append(name)
            shape = tuple(alloc.tensor_shape)
            dtype = mybir.dt.np(alloc.dtype)
            out_avals.append(jax.core.ShapedArray(shape, dtype))
            zero_outs.append(np.zeros(shape, dtype))
    n_params = len(in_names)
    all_in_names = list(in_names) + list(out_names)
    if partition_name is not None:
        all_in_names.append(partition_name)
    donate = tuple(range(n_params, n_params + len(out_names)))

    def _body(*args):
        operands = list(args)
        if partition_name is not None:
            operands.append(partition_id_tensor())
        outs = _bass_exec_p.bind(
            *operands, out_avals=tuple(out_avals), in_names=tuple(all_in_names),
            out_names=tuple(out_names), lowering_input_output_aliases=(),
            sim_require_finite=True, sim_require_nnan=True, nc=nc)
        return tuple(outs)

    fn = jax.jit(_body, donate_argnums=donate, keep_unused=True)
    futures = []
    for c, in_map in zip(core_ids, in_maps):
        dev = jax.devices()[c]
        args = [jax.device_put(np.asarray(in_map[nm]), dev) for nm in in_names]
        zz = [jax.device_put(z, dev) for z in zero_outs]
        futures.append(fn(*args, *zz))
    return [{nm: np.asarray(a) for nm, a in zip(out_names, f)} for f in futures]


_PROGRAM_CACHE = {}

MM_DT = "bf16"


def kernel(**inputs):
    os.environ.setdefault("NEURON_COMPILE_CACHE_URL", "/tmp/neuron_cache_kernel")
    x = np.asarray(inputs["x"], dtype=np.float32)
    mask = np.asarray(inputs["mask"]).astype(bool)
    pos_emb = np.asarray(inputs["pos_emb"], dtype=np.float32)
    g = np.asarray(inputs["g"], dtype=np.float32)
    Wq = np.asarray(inputs["Wq"], dtype=np.float32)
    Wkv = np.asarray(inputs["Wkv"], dtype=np.float32)
    Wo = np.asarray(inputs["Wo"], dtype=np.float32)
    bo = np.asarray(inputs["bo"], dtype=np.float32)
    b, n, _ = x.shape
    assert (b, n) == (2, 2048), (b, n)
    mm_dt = MM_DT
    use_km = not bool(mask.all())
    key = (n, mm_dt, use_km)
    if key not in _PROGRAM_CACHE:
        _PROGRAM_CACHE[key] = build_program(n=n, mm_dt=mm_dt, use_kmask=use_km)
    nc = _PROGRAM_CACHE[key]
    core_ids = list(range(8))
    in_maps = [make_core_inputs(x, mask, pos_emb, g, Wq, Wkv, Wo, c, n, mm_dt)
               for c in core_ids]
    results = _run_per_device(nc, in_maps, core_ids)
    out = np.zeros((b, n, D), np.float32)
    for c in core_ids:
        out[c // 4] += results[c]["out"].astype(np.float32)
    out += bo[None, None, :]
    return out


# revision 20
# speedup vs baseline: 1.4708x; 1.0153x over previous
"""Sharded causal attention kernel for trn2 (per-core program builder), v4.

Sharding: 8 cores = 2 batches x 4 head-groups (4 heads each).
v4 over v3:
  - attention g-loop software-pipelined: sims(g+1) issue before pv(g)
    so the Act engine runs exps back-to-back (Act-bound inner loop)
  - rmsnorm: squares split DVE/Act, abs_rsqrt on Act replaces
    sqrt+max+reciprocal; ssq packed in one PSUM bank ([4,512])
  - rotary: psr scaled in-place in PSUM, staged to SBUF by Act copy,
    merged into qT/kT via gpsimd DMA accumulate-adds (DVE freed)
  - weights loaded as one DMA per tensor (HWDGE is ~625ns/instr)
  - output staged per token-tile ([128,1024]) halving store DMAs
"""

from contextlib import ExitStack

import numpy as np

import concourse.bass as bass
import concourse.mybir as mybir
import concourse.tile as tile
from concourse import bacc

f32 = mybir.dt.float32
f32r = mybir.dt.float32r
bf16 = mybir.dt.bfloat16
AF = mybir.ActivationFunctionType
OP = mybir.AluOpType

D = 1024
HPC = 4
DH = 64
ROT = 32
P = 128
NEG = -1e30


def build_program(n=2048, mm_dt="bf16", use_kmask=False):
    KT = D // P
    NQB = n // 512
    NTOK = n // P
    NCH = n // 512
    mdt = {"f32": f32, "f32r": f32r, "bf16": bf16}[mm_dt]
    nc = bacc.Bacc("TRN2", target_bir_lowering=False, debug=False)

    def din(name, shape, dt_):
        return nc.dram_tensor(name, shape, dt_, kind="ExternalInput")

    xT_d = din("xT", [D, n], mdt)
    # weights come in t-major packed layout [128, KT*cols] (one DMA each)
    wq_d = din("wq", [P, KT * HPC * DH], mdt)
    wk_d = din("wk", [P, KT * HPC * DH], mdt)
    wv_d = din("wv", [P, KT * HPC * DH], mdt)
    wqr_d = din("wqr", [P, KT * P], mdt)   # 4 heads x 32 rot cols per t
    wkr_d = din("wkr", [P, KT * P], mdt)
    wo_d = din("wo", [P, 2 * D], mdt)
    cos_d = din("cos128", [P, n], f32)   # rot rows cos, pass rows 1.0
    sin_d = din("sinc128", [P, n], f32)  # all four 32-row blocks = sin
    tri_d = din("tri01", [P, P], mdt)    # 1.0 where key<=query else 0.0
    id_d = din("ident", [P, P], f32)
    km_d = din("kmask", [P, NTOK], f32) if use_kmask else None
    out_d = nc.dram_tensor("out", [n, D], mdt, kind="ExternalOutput")

    with tile.TileContext(nc) as tc, ExitStack() as top:
        persist = top.enter_context(tc.tile_pool(name="persist", bufs=1))
        ones_f32 = persist.tile([P, 1], f32, name="ones_f32")
        nc.vector.memset(ones_f32, 1.0)
        ones_col = persist.tile([P, 1], mdt, name="ones_col")
        nc.vector.tensor_copy(ones_col, ones_f32)
        ones_row_f = persist.tile([1, P], f32, name="ones_row_f")
        nc.vector.memset(ones_row_f, 1.0)
        ones_row = persist.tile([1, P], f32r, name="ones_row")
        nc.vector.tensor_copy(ones_row, ones_row_f)
        # preload the act table containing Square/AbsRsqrt/Copy so the norm
        # path doesn't eat a mid-phase table switch (Exp set loads later once)
        dummy_act = persist.tile([1, 1], f32, name="dummy_act")
        nc.scalar.activation(dummy_act, ones_f32[0:1, 0:1],
                             AF.Abs_reciprocal_sqrt)

        qkv = top.enter_context(tc.tile_pool(name="qkv", bufs=1))
        qT = [qkv.tile([P, n], mdt, name=f"qT{m}", tag=f"qT{m}") for m in range(2)]
        kT = [qkv.tile([P, n], mdt, name=f"kT{m}", tag=f"kT{m}") for m in range(2)]
        v_sb = [qkv.tile([P, HPC * (DH + 1)], mdt, name=f"v{tk}", tag=f"v{tk}")
                for tk in range(NTOK)]
        normk = top.enter_context(tc.tile_pool(name="normk", bufs=1))
        rs_col = normk.tile([P, NTOK], f32, name="rs_col")
        late = top.enter_context(tc.tile_pool(name="late", bufs=1))
        wop = top.enter_context(tc.tile_pool(name="wop", bufs=1))

        with ExitStack() as phase_a:
            big = phase_a.enter_context(tc.tile_pool(name="big", bufs=1))
            # DMA issue order = consumption order (single serialized DMA).
            x_sb = [big.tile([P, n], mdt, name=f"x{t}", tag=f"x{t}") for t in range(KT)]
            for t in range(KT):
                nc.sync.dma_start(out=x_sb[t], in_=xT_d[t * P:(t + 1) * P, :])

            def wload(dsrc, w_, nm):
                tl = big.tile([P, KT * w_], mdt, name=nm, tag=nm)
                nc.sync.dma_start(out=tl, in_=dsrc[:])
                return [tl[:, t * w_:(t + 1) * w_] for t in range(KT)]

            wq = wload(wq_d, HPC * DH, "wq")
            wk = wload(wk_d, HPC * DH, "wk")
            rot = phase_a.enter_context(tc.tile_pool(name="rot", bufs=1))
            cos_sb = rot.tile([P, n], f32, name="cos_sb")
            sin_sb = rot.tile([P, n], f32, name="sin_sb")
            nc.sync.dma_start(out=cos_sb, in_=cos_d[:])
            nc.sync.dma_start(out=sin_sb, in_=sin_d[:])
            wqr = wload(wqr_d, P, "wqr")
            wkr = wload(wkr_d, P, "wkr")
            wv = wload(wv_d, HPC * DH, "wv")
            wo_all = wop.tile([P, 2 * D], mdt, name="wo_all")
            nc.sync.dma_start(out=wo_all, in_=wo_d[:])
            wo_sb = [wo_all[:, m * D:(m + 1) * D] for m in range(2)]
            ident_sb = persist.tile([P, P], f32, name="ident_sb")
            nc.sync.dma_start(out=ident_sb, in_=id_d[:])
            tri_sb = persist.tile([P, P], mdt, name="tri_sb")
            nc.sync.dma_start(out=tri_sb, in_=tri_d[:])
            if use_kmask:
                km_sb = persist.tile([P, NTOK], f32, name="km_sb")
                nc.sync.dma_start(out=km_sb, in_=km_d[:])

            with tc.tile_pool(name="pproj", bufs=3, space="PSUM") as pp, \
                 tc.tile_pool(name="pnorm", bufs=1, space="PSUM") as pnorm, \
                 tc.tile_pool(name="pbc", bufs=1, space="PSUM") as pbc, \
                 tc.tile_pool(name="ppv", bufs=2, space="PSUM") as ppv, \
                 tc.tile_pool(name="normt", bufs=1) as normt, \
                 tc.tile_pool(name="sqp", bufs=2) as sqp, \
                 tc.tile_pool(name="usp", bufs=2) as usp:
                # ---- rmsnorm (overlaps x streaming; sq split DVE/Act) ----
                # matmul PSUM writes must start at partition 0/32/64: two
                # chunks per bank at partitions {0, 64}
                ssq2 = [pnorm.tile([P, 512], f32, name=f"ssq2_{i}", tag=f"ssq2_{i}")
                        for i in range(2)]
                sloc = [(ssq2[c // 2], 64 * (c % 2)) for c in range(NCH)]
                for t in range(KT):
                    for c in range(NCH):
                        sq = sqp.tile([P, 512], mdt, name=f"sq{t}_{c}", tag="sq")
                        xs = x_sb[t][:, c * 512:(c + 1) * 512]
                        if (t * NCH + c) % 2 == 0:
                            nc.vector.tensor_mul(sq, xs, xs)
                        else:
                            nc.scalar.activation(sq, xs, AF.Square)
                        stile, soff = sloc[c]
                        nc.tensor.matmul(stile[soff:soff + 1, :], ones_col, sq,
                                         start=(t == 0), stop=(t == KT - 1))
                s_row = normt.tile([1, n], f32r, name="s_row")
                for c in range(NCH):
                    sl = slice(c * 512, (c + 1) * 512)
                    stile, soff = sloc[c]
                    # s = 1/sqrt(ssq/D); matches 1/max(sqrt(.), eps) for all
                    # realistic (nonzero) token rows
                    with nc.allow_low_precision(reason="f32r has f32 bits"):
                        nc.scalar.activation(s_row[:, sl], stile[soff:soff + 1, :],
                                             AF.Abs_reciprocal_sqrt, scale=1.0 / D)
                    bc = pbc.tile([P, 512], f32, name=f"bc{c}", tag="bc")
                    nc.tensor.matmul(bc, ones_row, s_row[:, sl],
                                     start=True, stop=True)
                    nc.vector.tensor_mul(cos_sb[:, sl], cos_sb[:, sl], bc)
                    nc.vector.tensor_mul(sin_sb[:, sl], sin_sb[:, sl], bc)
                    for tb in range(4):
                        tk = c * 4 + tb
                        dg = sqp.tile([P, P], f32, name=f"dg_{tk}", tag="dg")
                        nc.vector.tensor_mul(dg, bc[:, tb * P:(tb + 1) * P], ident_sb)
                        nc.vector.reduce_sum(rs_col[:, tk:tk + 1], dg,
                                             axis=mybir.AxisListType.X)

                # ---- q projection, then v, then k: phase-B PSUM pools
                # reuse these banks, so the last readers must not be the
                # v-scale tail (q,v,k order lets attention start unstalled)
                def proj_qk(base, wmain, wrot, nm_):
                    for c in range(NCH):
                        sl = slice(c * 512, (c + 1) * 512)
                        pss = []
                        for m in range(2):
                            ps = pp.tile([P, 512], f32,
                                         name=f"pp{nm_}{m}_{c}", tag="pp")
                            for t in range(KT):
                                nc.tensor.matmul(
                                    ps, wmain[t][:, m * P:(m + 1) * P],
                                    x_sb[t][:, sl],
                                    start=(t == 0), stop=(t == KT - 1))
                            pss.append(ps)
                        for m in range(2):
                            nc.vector.tensor_mul(base[m][:, sl], pss[m],
                                                 cos_sb[:, sl])
                        psr = pp.tile([P, 512], f32, name=f"pp{nm_}r_{c}", tag="pp")
                        for t in range(KT):
                            nc.tensor.matmul(psr, wrot[t], x_sb[t][:, sl],
                                             start=(t == 0), stop=(t == KT - 1))
                        nc.vector.tensor_mul(psr, psr, sin_sb[:, sl])
                        u = usp.tile([P, 512], mdt, name=f"u_{nm_}{c}", tag="u")
                        nc.scalar.copy(u, psr)
                        for h in range(HPC):
                            m, h2 = h // 2, h % 2
                            nc.gpsimd.dma_start(
                                out=base[m][64 * h2:64 * h2 + ROT, sl],
                                in_=u[ROT * h:ROT * (h + 1), :],
                                accum_op=OP.add)

                proj_qk(qT, wq, wqr, "q")
                # ---- v projection ----
                for tk in range(NTOK):
                    ps = ppv.tile([P, HPC * DH], f32, name=f"ppv_{tk}", tag="ppv")
                    for t in range(KT):
                        nc.tensor.matmul(ps, x_sb[t][:, tk * P:(tk + 1) * P], wv[t],
                                         start=(t == 0), stop=(t == KT - 1))
                    vv = v_sb[tk].rearrange("p (h c) -> p h c", h=HPC)
                    nc.vector.tensor_scalar_mul(
                        vv[:, :, 0:DH], ps.rearrange("p (h c) -> p h c", h=HPC),
                        rs_col[:, tk:tk + 1])
                    for hh in range(HPC):
                        nc.vector.tensor_copy(vv[:, hh, DH:DH + 1], ones_col)
                proj_qk(kT, wk, wkr, "k")

        # ---- attention (software-pipelined) + output projection ----
        with tc.tile_pool(name="ep", bufs=2) as ep, \
             tc.tile_pool(name="rbp", bufs=2) as rbp, \
             tc.tile_pool(name="bcdp", bufs=2) as bcdp, \
             tc.tile_pool(name="outsb", bufs=4) as osb, \
             tc.tile_pool(name="psim", bufs=1, space="PSUM") as psim, \
             tc.tile_pool(name="pmix", bufs=4, space="PSUM") as pmix:
            for qb in range(NQB):
                nkt = 4 * qb + 4
                ng = (nkt + 1) // 2
                qsl = slice(qb * 512, (qb + 1) * 512)
                attn = [late.tile([P, 512], mdt, name=f"attn{pr}_{qb}",
                                  tag=f"attn{pr}") for pr in range(2)]
                for pr in range(2):
                    pvh = [pmix.tile([DH + 1, 512], f32, name=f"pv_{pr}_{qb}_{h2}",
                                     tag="b512") for h2 in range(2)]

                    def emit_sims(g):
                        # per-tile trimmed segments: diagonal tile kt only
                        # needs q columns >= (kt-4qb)*128 (rest fully masked)
                        segs, off = [], 0
                        for kt_ in (2 * g, 2 * g + 1):
                            if kt_ >= nkt:
                                continue
                            qlo = max(0, kt_ - 4 * qb) * P
                            segs.append((kt_, qlo, off, 512 - qlo))
                            off += 512 - qlo
                        sims = [psim.tile([P, off], f32, name=f"s{h2}_{pr}_{qb}_{g}",
                                          tag=f"sim{h2}") for h2 in range(2)]
                        for kt_, qlo, o, w in segs:
                            for h2 in range(2):
                                nc.tensor.matmul(
                                    sims[h2][:, o:o + w],
                                    kT[pr][64 * h2:64 * h2 + 64,
                                           kt_ * P:(kt_ + 1) * P],
                                    qT[pr][64 * h2:64 * h2 + 64,
                                           qb * 512 + qlo:(qb + 1) * 512],
                                    start=True, stop=True,
                                    tile_position=(64 * h2, 0))
                        if use_kmask:
                            for kt_, qlo, o, w in segs:
                                for h2 in range(2):
                                    sl = sims[h2][:, o:o + w]
                                    nc.vector.tensor_scalar_add(
                                        sl, sl, km_sb[:, kt_:kt_ + 1])
                        return sims, segs

                    cur = emit_sims(0)
                    for g in range(ng):
                        sims, segs = cur
                        w_ = segs[-1][2] + segs[-1][3]
                        Es = [ep.tile([P, w_], mdt, name=f"E{h2}_{pr}_{qb}_{g}",
                                      tag=f"E{h2}") for h2 in range(2)]
                        for h2 in range(2):
                            nc.scalar.activation(Es[h2], sims[h2], AF.Exp)
                        for kt_, qlo, o, w in segs:
                            if kt_ - 4 * qb >= 0:
                                for h2 in range(2):
                                    sl = Es[h2][:, o:o + P]
                                    nc.vector.tensor_mul(sl, sl, tri_sb)
                        if g + 1 < ng:
                            cur = emit_sims(g + 1)
                        for kt_, qlo, o, w in segs:
                            for h2 in range(2):
                                hh = 2 * pr + h2
                                nc.tensor.matmul(
                                    pvh[h2][:, qlo:512],
                                    v_sb[kt_][:, (DH + 1) * hh:
                                              (DH + 1) * hh + DH + 1],
                                    Es[h2][:, o:o + w],
                                    start=(kt_ == 0), stop=(kt_ == nkt - 1),
                                    skip_group_check=True)
                    for h2 in range(2):
                        rb = rbp.tile([1, 512], f32r, name=f"rb_{pr}_{qb}_{h2}",
                                      tag="rb")
                        with nc.allow_low_precision(reason="f32r has f32 bits"):
                            nc.vector.reciprocal(rb, pvh[h2][DH:DH + 1, :])
                        bps = pmix.tile([DH, 512], f32, name=f"bps_{pr}_{qb}_{h2}",
                                        tag="b512")
                        nc.tensor.matmul(bps, ones_row[:, 0:DH], rb,
                                         start=True, stop=True)
                        bcd = bcdp.tile([DH, 512], f32, name=f"bcd_{pr}_{qb}_{h2}",
                                        tag="bcd")
                        nc.vector.tensor_copy(bcd, bps)
                        nc.vector.tensor_tensor(
                            attn[pr][64 * h2:64 * h2 + 64, :], pvh[h2][0:DH, :],
                            bcd, OP.mult)
                    if qb == NQB - 1 and pr == 0:
                        # last q-block: start the m=0 half of the out-proj
                        # during pr=1's attention to shorten the drain tail
                        obs = {}
                        for tk in range(4 * qb, 4 * qb + 4):
                            tkl = tk - 4 * qb
                            ob = osb.tile([P, D], mdt, name=f"ob_{tk}", tag="ob")
                            obs[tk] = ob
                            for c2 in range(D // 512):
                                po = pmix.tile([P, 512], f32,
                                               name=f"po0_{tk}_{c2}", tag="b512")
                                nc.tensor.matmul(
                                    po, attn[0][:, tkl * P:(tkl + 1) * P],
                                    wo_sb[0][:, c2 * 512:(c2 + 1) * 512],
                                    start=True, stop=True)
                                nc.vector.tensor_copy(
                                    ob[:, c2 * 512:(c2 + 1) * 512], po)
                # output projection for this q-block's token rows
                for tk in range(4 * qb, 4 * qb + 4):
                    tkl = tk - 4 * qb
                    if qb == NQB - 1:
                        ob = obs[tk]
                        for c2 in range(D // 512):
                            po = pmix.tile([P, 512], f32, name=f"po1_{tk}_{c2}",
                                           tag="b512")
                            nc.tensor.matmul(
                                po, attn[1][:, tkl * P:(tkl + 1) * P],
                                wo_sb[1][:, c2 * 512:(c2 + 1) * 512],
                                start=True, stop=True)
                            obc = ob[:, c2 * 512:(c2 + 1) * 512]
                            nc.vector.tensor_tensor(obc, obc, po, OP.add)
                    else:
                        ob = osb.tile([P, D], mdt, name=f"ob_{tk}", tag="ob")
                        for c2 in range(D // 512):
                            po = pmix.tile([P, 512], f32, name=f"po_{tk}_{c2}",
                                           tag="b512")
                            for m in range(2):
                                nc.tensor.matmul(
                                    po, attn[m][:, tkl * P:(tkl + 1) * P],
                                    wo_sb[m][:, c2 * 512:(c2 + 1) * 512],
                                    start=(m == 0), stop=(m == 1))
                            nc.vector.tensor_copy(ob[:, c2 * 512:(c2 + 1) * 512],
                                                  po)
                    nc.sync.dma_start(out=out_d[tk * P:(tk + 1) * P, :], in_=ob)

    nc.compile()
    return nc


# ---------------------------------------------------------------- host side

def np_dt(mm_dt):
    import ml_dtypes
    return {"f32": np.float32, "f32r": np.float32, "bf16": ml_dtypes.bfloat16}[mm_dt]


def _tmajor(W):
    """[D, cols] -> [128, KT*cols] t-major packing for single-DMA load."""
    KT = W.shape[0] // P
    return np.concatenate([W[t * P:(t + 1) * P, :] for t in range(KT)], axis=1)


def make_core_inputs(x, mask, pos_emb, g, Wq, Wkv, Wo, core, n, mm_dt="bf16"):
    ndt = np_dt(mm_dt)
    b = core // 4
    h0 = (core % 4) * HPC
    scale = DH ** -0.5
    gW = Wq * g[:, None]
    gKV = Wkv * g[:, None]
    cols = slice(h0 * DH, (h0 + HPC) * DH)
    wq = gW[:, cols] * scale
    Wk_full = gKV[:, :D]
    Wv_full = gKV[:, D:]
    wk = Wk_full[:, cols]
    wv = Wv_full[:, cols]

    def rot_cols(W):
        # [h0:32 | h1:32 | h2:32 | h3:32] rotate-half columns
        out = np.zeros((D, P), dtype=W.dtype)
        for h in range(HPC):
            src = W[:, (h0 + h) * DH:(h0 + h) * DH + DH]
            base = h * ROT
            out[:, base:base + 16] = -src[:, 16:32]
            out[:, base + 16:base + 32] = src[:, 0:16]
        return out

    wqr = rot_cols(gW) * scale
    wkr = rot_cols(Wk_full)
    wo = np.concatenate([Wo[cols, :][m * P:(m + 1) * P, :] for m in range(2)],
                        axis=1)

    cosf = np.cos(pos_emb.T).astype(np.float32)
    sinf = np.sin(pos_emb.T).astype(np.float32)
    cos128 = np.ones((P, n), np.float32)
    cos128[0:ROT] = cosf
    cos128[DH:DH + ROT] = cosf
    sinc128 = np.empty((P, n), np.float32)
    for h in range(HPC):
        sinc128[h * ROT:(h + 1) * ROT] = sinf
    tri01 = (np.arange(P)[:, None] <= np.arange(P)[None, :]).astype(np.float32)

    ins = {
        "xT": np.ascontiguousarray(x[b].T).astype(ndt),
        "wq": _tmajor(wq).astype(ndt), "wk": _tmajor(wk).astype(ndt),
        "wv": _tmajor(wv).astype(ndt), "wqr": _tmajor(wqr).astype(ndt),
        "wkr": _tmajor(wkr).astype(ndt), "wo": wo.astype(ndt),
        "cos128": cos128, "sinc128": sinc128, "tri01": tri01.astype(ndt),
        "ident": np.eye(P, dtype=np.float32),
    }
    if not mask.all():
        km = np.where(mask[b], 0.0, NEG).astype(np.float32)
        ins["kmask"] = np.ascontiguousarray(km.reshape(n // P, P).T)
    return ins


# ---------------------------------------------------------------- runner

import os
import jax


def _run_per_device(nc, in_maps, core_ids):
    """Run the same Bass program independently on each visible device."""
    from concourse.bass2jax import (_bass_exec_p, install_neuronx_cc_hook,
                                    partition_id_tensor)
    install_neuronx_cc_hook()
    partition_name = nc.partition_id_tensor.name if nc.partition_id_tensor else None
    in_names, out_names, out_avals, zero_outs = [], [], [], []
    for alloc in nc.m.functions[0].allocations:
        if not isinstance(alloc, mybir.MemoryLocationSet):
            continue
        name = alloc.memorylocations[0].name
        if alloc.kind == "ExternalInput":
            if name != partition_name:
                in_names.append(name)
        elif alloc.kind == "ExternalOutput":
            out_names.append(name)
            shape = tuple(alloc.tensor_shape)
            dtype = mybir.dt.np(alloc.dtype)
            out_avals.append(jax.core.ShapedArray(shape, dtype))
            zero_outs.append(np.zeros(shape, dtype))
    n_params = len(in_names)
    all_in_names = list(in_names) + list(out_names)
    if partition_name is not None:
        all_in_names.append(partition_name)
    donate = tuple(range(n_params, n_params + len(out_names)))

    def _body(*args):
        operands = list(args)
        if partition_name is not None:
            operands.append(partition_id_tensor())
        outs = _bass_exec_p.bind(
            *operands, out_avals=tuple(out_avals), in_names=tuple(all_in_names),
            out_names=tuple(out_names), lowering_input_output_aliases=(),
            sim_require_finite=True, sim_require_nnan=True, nc=nc)
        return tuple(outs)

    fn = jax.jit(_body, donate_argnums=donate, keep_unused=True)
    futures = []
    for c, in_map in zip(core_ids, in_maps):
        dev = jax.devices()[c]
        args = [jax.device_put(np.asarray(in_map[nm]), dev) for nm in in_names]
        zz = [jax.device_put(z, dev) for z in zero_outs]
        futures.append(fn(*args, *zz))
    return [{nm: np.asarray(a) for nm, a in zip(out_names, f)} for f in futures]


_PROGRAM_CACHE = {}

MM_DT = "bf16"


def kernel(**inputs):
    os.environ.setdefault("NEURON_COMPILE_CACHE_URL", "/tmp/neuron_cache_kernel")
    x = np.asarray(inputs["x"], dtype=np.float32)
    mask = np.asarray(inputs["mask"]).astype(bool)
    pos_emb = np.asarray(inputs["pos_emb"], dtype=np.float32)
    g = np.asarray(inputs["g"], dtype=np.float32)
    Wq = np.asarray(inputs["Wq"], dtype=np.float32)
    Wkv = np.asarray(inputs["Wkv"], dtype=np.float32)
    Wo = np.asarray(inputs["Wo"], dtype=np.float32)
    bo = np.asarray(inputs["bo"], dtype=np.float32)
    b, n, _ = x.shape
    assert (b, n) == (2, 2048), (b, n)
    mm_dt = MM_DT
    use_km = not bool(mask.all())
    key = (n, mm_dt, use_km)
    if key not in _PROGRAM_CACHE:
        _PROGRAM_CACHE[key] = build_program(n=n, mm_dt=mm_dt, use_kmask=use_km)
    nc = _PROGRAM_CACHE[key]
    core_ids = list(range(8))
    in_maps = [make_core_inputs(x, mask, pos_emb, g, Wq, Wkv, Wo, c, n, mm_dt)
               for c in core_ids]
    results = _run_per_device(nc, in_maps, core_ids)
    out = np.zeros((b, n, D), np.float32)
    for c in core_ids:
        out[c // 4] += results[c]["out"].astype(np.float32)
    out += bo[None, None, :]
    return out


# revision 23
# speedup vs baseline: 1.5550x; 1.0572x over previous
"""Sharded causal attention kernel for trn2 (per-core program builder), v4.

Sharding: 8 cores = 2 batches x 4 head-groups (4 heads each).
v4 over v3:
  - attention g-loop software-pipelined: sims(g+1) issue before pv(g)
    so the Act engine runs exps back-to-back (Act-bound inner loop)
  - rmsnorm: squares split DVE/Act, abs_rsqrt on Act replaces
    sqrt+max+reciprocal; ssq packed in one PSUM bank ([4,512])
  - rotary: psr scaled in-place in PSUM, staged to SBUF by Act copy,
    merged into qT/kT via gpsimd DMA accumulate-adds (DVE freed)
  - weights loaded as one DMA per tensor (HWDGE is ~625ns/instr)
  - output staged per token-tile ([128,1024]) halving store DMAs
"""

from contextlib import ExitStack

import numpy as np

import concourse.bass as bass
import concourse.mybir as mybir
import concourse.tile as tile
from concourse import bacc

f32 = mybir.dt.float32
f32r = mybir.dt.float32r
bf16 = mybir.dt.bfloat16
AF = mybir.ActivationFunctionType
OP = mybir.AluOpType

D = 1024
HPC = 4
DH = 64
ROT = 32
P = 128
NEG = -1e30


def build_program(n=2048, mm_dt="bf16", use_kmask=False):
    KT = D // P
    NQB = n // 512
    NTOK = n // P
    NCH = n // 512
    mdt = {"f32": f32, "f32r": f32r, "bf16": bf16}[mm_dt]
    nc = bacc.Bacc("TRN2", target_bir_lowering=False, debug=False)

    def din(name, shape, dt_):
        return nc.dram_tensor(name, shape, dt_, kind="ExternalInput")

    xT_d = din("xT", [D, n], mdt)
    # weights come in t-major packed layout [128, KT*cols] (one DMA each)
    wq_d = din("wq", [P, KT * HPC * DH], mdt)
    wk_d = din("wk", [P, KT * HPC * DH], mdt)
    wv_d = din("wv", [P, KT * HPC * DH], mdt)
    wqr_d = din("wqr", [P, KT * P], mdt)   # 4 heads x 32 rot cols per t
    wkr_d = din("wkr", [P, KT * P], mdt)
    wo_d = din("wo", [P, 2 * D], mdt)
    cos_d = din("cos128", [P, n], f32)   # rot rows cos, pass rows 1.0
    sin_d = din("sinc128", [P, n], f32)  # all four 32-row blocks = sin
    tri_d = din("tri01", [P, P], mdt)    # 1.0 where key<=query else 0.0
    id_d = din("ident", [P, P], f32)
    km_d = din("kmask", [P, NTOK], f32) if use_kmask else None
    out_d = nc.dram_tensor("out", [n, D], mdt, kind="ExternalOutput")

    with tile.TileContext(nc) as tc, ExitStack() as top:
        persist = top.enter_context(tc.tile_pool(name="persist", bufs=1))
        ones_f32 = persist.tile([P, 1], f32, name="ones_f32")
        nc.vector.memset(ones_f32, 1.0)
        ones_col = persist.tile([P, 1], mdt, name="ones_col")
        nc.vector.tensor_copy(ones_col, ones_f32)
        ones_row_f = persist.tile([1, P], f32, name="ones_row_f")
        nc.vector.memset(ones_row_f, 1.0)
        ones_row = persist.tile([1, P], f32r, name="ones_row")
        nc.vector.tensor_copy(ones_row, ones_row_f)
        # preload the act table containing Square/AbsRsqrt/Copy so the norm
        # path doesn't eat a mid-phase table switch (Exp set loads later once)
        dummy_act = persist.tile([1, 1], f32, name="dummy_act")
        nc.scalar.activation(dummy_act, ones_f32[0:1, 0:1],
                             AF.Abs_reciprocal_sqrt)

        qkv = top.enter_context(tc.tile_pool(name="qkv", bufs=1))
        qT = [qkv.tile([P, n], mdt, name=f"qT{m}", tag=f"qT{m}") for m in range(2)]
        kT = [qkv.tile([P, n], mdt, name=f"kT{m}", tag=f"kT{m}") for m in range(2)]
        # per head: [64 v-dims | 64 ones]; the ones block makes the pv matmul
        # emit the softmax denominator replicated on psum rows 64:128
        v_sb = [qkv.tile([P, HPC * 2 * DH], mdt, name=f"v{tk}", tag=f"v{tk}")
                for tk in range(NTOK)]
        for tk in range(NTOK):
            vv = v_sb[tk].rearrange("p (h c) -> p h c", h=HPC)
            for hh in range(HPC):
                nc.gpsimd.memset(vv[:, hh, DH:2 * DH], 1.0)
        normk = top.enter_context(tc.tile_pool(name="normk", bufs=1))
        rs_col = normk.tile([P, NTOK], f32, name="rs_col")
        late = top.enter_context(tc.tile_pool(name="late", bufs=1))
        wop = top.enter_context(tc.tile_pool(name="wop", bufs=1))

        with ExitStack() as phase_a:
            big = phase_a.enter_context(tc.tile_pool(name="big", bufs=1))
            # DMA issue order = consumption order (single serialized DMA).
            x_sb = [big.tile([P, n], mdt, name=f"x{t}", tag=f"x{t}") for t in range(KT)]
            for t in range(KT):
                nc.sync.dma_start(out=x_sb[t], in_=xT_d[t * P:(t + 1) * P, :])

            def wload(dsrc, w_, nm):
                tl = big.tile([P, KT * w_], mdt, name=nm, tag=nm)
                nc.sync.dma_start(out=tl, in_=dsrc[:])
                return [tl[:, t * w_:(t + 1) * w_] for t in range(KT)]

            wq = wload(wq_d, HPC * DH, "wq")
            wk = wload(wk_d, HPC * DH, "wk")
            rot = phase_a.enter_context(tc.tile_pool(name="rot", bufs=1))
            cos_sb = rot.tile([P, n], f32, name="cos_sb")
            sin_sb = rot.tile([P, n], f32, name="sin_sb")
            nc.sync.dma_start(out=cos_sb, in_=cos_d[:])
            nc.sync.dma_start(out=sin_sb, in_=sin_d[:])
            wqr = wload(wqr_d, P, "wqr")
            wkr = wload(wkr_d, P, "wkr")
            wv = wload(wv_d, HPC * DH, "wv")
            wo_all = wop.tile([P, 2 * D], mdt, name="wo_all")
            nc.sync.dma_start(out=wo_all, in_=wo_d[:])
            wo_sb = [wo_all[:, m * D:(m + 1) * D] for m in range(2)]
            ident_sb = persist.tile([P, P], f32, name="ident_sb")
            nc.sync.dma_start(out=ident_sb, in_=id_d[:])
            tri_sb = persist.tile([P, P], mdt, name="tri_sb")
            nc.sync.dma_start(out=tri_sb, in_=tri_d[:])
            if use_kmask:
                km_sb = persist.tile([P, NTOK], f32, name="km_sb")
                nc.sync.dma_start(out=km_sb, in_=km_d[:])

            with tc.tile_pool(name="pproj", bufs=3, space="PSUM") as pp, \
                 tc.tile_pool(name="pnorm", bufs=1, space="PSUM") as pnorm, \
                 tc.tile_pool(name="pbc", bufs=1, space="PSUM") as pbc, \
                 tc.tile_pool(name="ppv", bufs=2, space="PSUM") as ppv, \
                 tc.tile_pool(name="normt", bufs=1) as normt, \
                 tc.tile_pool(name="sqp", bufs=2) as sqp, \
                 tc.tile_pool(name="usp", bufs=2) as usp:
                # ---- rmsnorm (overlaps x streaming; sq split DVE/Act) ----
                # matmul PSUM writes must start at partition 0/32/64: two
                # chunks per bank at partitions {0, 64}
                ssq2 = [pnorm.tile([P, 512], f32, name=f"ssq2_{i}", tag=f"ssq2_{i}")
                        for i in range(2)]
                sloc = [(ssq2[c // 2], 64 * (c % 2)) for c in range(NCH)]
                for t in range(KT):
                    for c in range(NCH):
                        sq = sqp.tile([P, 512], mdt, name=f"sq{t}_{c}", tag="sq")
                        xs = x_sb[t][:, c * 512:(c + 1) * 512]
                        if (t * NCH + c) % 2 == 0:
                            nc.vector.tensor_mul(sq, xs, xs)
                        else:
                            nc.scalar.activation(sq, xs, AF.Square)
                        stile, soff = sloc[c]
                        nc.tensor.matmul(stile[soff:soff + 1, :], ones_col, sq,
                                         start=(t == 0), stop=(t == KT - 1))
                s_row = normt.tile([1, n], f32r, name="s_row")
                for c in range(NCH):
                    sl = slice(c * 512, (c + 1) * 512)
                    stile, soff = sloc[c]
                    # s = 1/sqrt(ssq/D); matches 1/max(sqrt(.), eps) for all
                    # realistic (nonzero) token rows
                    with nc.allow_low_precision(reason="f32r has f32 bits"):
                        nc.scalar.activation(s_row[:, sl], stile[soff:soff + 1, :],
                                             AF.Abs_reciprocal_sqrt, scale=1.0 / D)
                    bc = pbc.tile([P, 512], f32, name=f"bc{c}", tag="bc")
                    nc.tensor.matmul(bc, ones_row, s_row[:, sl],
                                     start=True, stop=True)
                    nc.vector.tensor_mul(cos_sb[:, sl], cos_sb[:, sl], bc)
                    nc.vector.tensor_mul(sin_sb[:, sl], sin_sb[:, sl], bc)
                    for tb in range(4):
                        tk = c * 4 + tb
                        dg = sqp.tile([P, P], f32, name=f"dg_{tk}", tag="dg")
                        nc.vector.tensor_mul(dg, bc[:, tb * P:(tb + 1) * P], ident_sb)
                        nc.vector.reduce_sum(rs_col[:, tk:tk + 1], dg,
                                             axis=mybir.AxisListType.X)

                # ---- q projection, then v, then k: phase-B PSUM pools
                # reuse these banks, so the last readers must not be the
                # v-scale tail (q,v,k order lets attention start unstalled)
                def proj_qk(base, wmain, wrot, nm_):
                    for c in range(NCH):
                        sl = slice(c * 512, (c + 1) * 512)
                        pss = []
                        for m in range(2):
                            ps = pp.tile([P, 512], f32,
                                         name=f"pp{nm_}{m}_{c}", tag="pp")
                            for t in range(KT):
                                nc.tensor.matmul(
                                    ps, wmain[t][:, m * P:(m + 1) * P],
                                    x_sb[t][:, sl],
                                    start=(t == 0), stop=(t == KT - 1))
                            pss.append(ps)
                        for m in range(2):
                            nc.vector.tensor_mul(base[m][:, sl], pss[m],
                                                 cos_sb[:, sl])
                        psr = pp.tile([P, 512], f32, name=f"pp{nm_}r_{c}", tag="pp")
                        for t in range(KT):
                            nc.tensor.matmul(psr, wrot[t], x_sb[t][:, sl],
                                             start=(t == 0), stop=(t == KT - 1))
                        nc.vector.tensor_mul(psr, psr, sin_sb[:, sl])
                        u = usp.tile([P, 512], mdt, name=f"u_{nm_}{c}", tag="u")
                        nc.scalar.copy(u, psr)
                        for h in range(HPC):
                            m, h2 = h // 2, h % 2
                            nc.gpsimd.dma_start(
                                out=base[m][64 * h2:64 * h2 + ROT, sl],
                                in_=u[ROT * h:ROT * (h + 1), :],
                                accum_op=OP.add)

                proj_qk(qT, wq, wqr, "q")
                # ---- v projection ----
                for tk in range(NTOK):
                    ps = ppv.tile([P, HPC * DH], f32, name=f"ppv_{tk}", tag="ppv")
                    for t in range(KT):
                        nc.tensor.matmul(ps, x_sb[t][:, tk * P:(tk + 1) * P], wv[t],
                                         start=(t == 0), stop=(t == KT - 1))
                    vv = v_sb[tk].rearrange("p (h c) -> p h c", h=HPC)
                    nc.vector.tensor_scalar_mul(
                        vv[:, :, 0:DH], ps.rearrange("p (h c) -> p h c", h=HPC),
                        rs_col[:, tk:tk + 1])
                proj_qk(kT, wk, wkr, "k")

        # ---- attention: one flat pipeline across all (qb, pr) units ----
        # exps(step) -> sims(step+1) -> pvs(step) keeps Act saturated even
        # across unit boundaries; unit tails (recip/attn-mul/out-proj) hang
        # off the side without stalling the exp stream.
        with tc.tile_pool(name="ep", bufs=2) as ep, \
             tc.tile_pool(name="rcpp", bufs=2) as rcpp, \
             tc.tile_pool(name="outsb", bufs=4) as osb, \
             tc.tile_pool(name="psim", bufs=1, space="PSUM") as psim, \
             tc.tile_pool(name="pvp", bufs=4, space="PSUM") as pvp:
            seq = [(qb, pr) for qb in (0, 3, 2, 1) for pr in range(2)]
            steps = [(qb, pr, g) for (qb, pr) in seq
                     for g in range((4 * qb + 4 + 1) // 2)]
            pvhs = {}
            attns = {}

            def emit_sims(qb, pr, g):
                nkt = 4 * qb + 4
                segs, off = [], 0
                for kt_ in (2 * g, 2 * g + 1):
                    if kt_ >= nkt:
                        continue
                    qlo = max(0, kt_ - 4 * qb) * P
                    segs.append((kt_, qlo, off, 512 - qlo))
                    off += 512 - qlo
                sims = [psim.tile([P, off], f32, name=f"s{h2}_{pr}_{qb}_{g}",
                                  tag=f"sim{h2}") for h2 in range(2)]
                for kt_, qlo, o, w in segs:
                    for h2 in range(2):
                        nc.tensor.matmul(
                            sims[h2][:, o:o + w],
                            kT[pr][64 * h2:64 * h2 + 64, kt_ * P:(kt_ + 1) * P],
                            qT[pr][64 * h2:64 * h2 + 64,
                                   qb * 512 + qlo:(qb + 1) * 512],
                            start=True, stop=True, tile_position=(64 * h2, 0))
                if use_kmask:
                    for kt_, qlo, o, w in segs:
                        for h2 in range(2):
                            sl = sims[h2][:, o:o + w]
                            nc.vector.tensor_scalar_add(sl, sl,
                                                        km_sb[:, kt_:kt_ + 1])
                return sims, segs

            def alloc_pvh(qb, pr):
                pvhs[(qb, pr)] = [
                    pvp.tile([P, 512], f32, name=f"pv_{pr}_{qb}_{h2}", tag="pv")
                    for h2 in range(2)]

            def emit_tail(qb, pr):
                pvh = pvhs[(qb, pr)]
                attns.setdefault(qb, [None, None])
                attns[qb][pr] = late.tile([P, 512], mdt, name=f"attn{pr}_{qb}",
                                          tag=f"attn{pr}")
                at = attns[qb][pr]
                for h2 in range(2):
                    rcp = rcpp.tile([DH, 512], f32, name=f"rcp_{pr}_{qb}_{h2}",
                                    tag="rcp")
                    nc.vector.reciprocal(rcp, pvh[h2][DH:2 * DH, :])
                    nc.vector.tensor_tensor(
                        at[64 * h2:64 * h2 + 64, :], pvh[h2][0:DH, :],
                        rcp, OP.mult)
                if pr == 1:
                    attn = attns[qb]
                    for tk in range(4 * qb, 4 * qb + 4):
                        tkl = tk - 4 * qb
                        ob = osb.tile([P, D], mdt, name=f"ob_{tk}", tag="ob")
                        for c2 in range(D // 512):
                            po = pvp.tile([P, 512], f32, name=f"po_{tk}_{c2}",
                                          tag="pv")
                            for m in range(2):
                                nc.tensor.matmul(
                                    po, attn[m][:, tkl * P:(tkl + 1) * P],
                                    wo_sb[m][:, c2 * 512:(c2 + 1) * 512],
                                    start=(m == 0), stop=(m == 1))
                            nc.vector.tensor_copy(ob[:, c2 * 512:(c2 + 1) * 512],
                                                  po)
                        nc.sync.dma_start(out=out_d[tk * P:(tk + 1) * P, :],
                                          in_=ob)

            alloc_pvh(*seq[0])
            cur = emit_sims(*steps[0])
            for idx, (qb, pr, g) in enumerate(steps):
                nkt = 4 * qb + 4
                ng = (nkt + 1) // 2
                sims, segs = cur
                w_ = segs[-1][2] + segs[-1][3]
                Es = [ep.tile([P, w_], mdt, name=f"E{h2}_{pr}_{qb}_{g}",
                              tag=f"E{h2}") for h2 in range(2)]
                for h2 in range(2):
                    nc.scalar.activation(Es[h2], sims[h2], AF.Exp)
                for kt_, qlo, o, w in segs:
                    if kt_ - 4 * qb >= 0:
                        for h2 in range(2):
                            sl = Es[h2][:, o:o + P]
                            nc.vector.tensor_mul(sl, sl, tri_sb)
                if idx + 1 < len(steps):
                    nqb, npr, ng_ = steps[idx + 1]
                    cur = emit_sims(nqb, npr, ng_)
                    if ng_ == 1 and 4 * nqb + 4 > 2:
                        # unit with >1 groups: pvh for it allocated at its g=1
                        # step top would be late; alloc after its first sims
                        pass
                if idx + 1 < len(steps) and steps[idx + 1][2] == 0:
                    alloc_pvh(steps[idx + 1][0], steps[idx + 1][1])
                pvh = pvhs[(qb, pr)]
                for kt_, qlo, o, w in segs:
                    for h2 in range(2):
                        hh = 2 * pr + h2
                        nc.tensor.matmul(
                            pvh[h2][:, qlo:512],
                            v_sb[kt_][:, 2 * DH * hh:2 * DH * hh + 2 * DH],
                            Es[h2][:, o:o + w],
                            start=(kt_ == 0), stop=(kt_ == nkt - 1),
                            skip_group_check=True)
                if g == ng - 1:
                    emit_tail(qb, pr)

    nc.compile()
    return nc


# ---------------------------------------------------------------- host side

def np_dt(mm_dt):
    import ml_dtypes
    return {"f32": np.float32, "f32r": np.float32, "bf16": ml_dtypes.bfloat16}[mm_dt]


def _tmajor(W):
    """[D, cols] -> [128, KT*cols] t-major packing for single-DMA load."""
    KT = W.shape[0] // P
    return np.concatenate([W[t * P:(t + 1) * P, :] for t in range(KT)], axis=1)


def make_core_inputs(x, mask, pos_emb, g, Wq, Wkv, Wo, core, n, mm_dt="bf16"):
    ndt = np_dt(mm_dt)
    b = core // 4
    h0 = (core % 4) * HPC
    scale = DH ** -0.5
    gW = Wq * g[:, None]
    gKV = Wkv * g[:, None]
    cols = slice(h0 * DH, (h0 + HPC) * DH)
    wq = gW[:, cols] * scale
    Wk_full = gKV[:, :D]
    Wv_full = gKV[:, D:]
    wk = Wk_full[:, cols]
    wv = Wv_full[:, cols]

    def rot_cols(W):
        # [h0:32 | h1:32 | h2:32 | h3:32] rotate-half columns
        out = np.zeros((D, P), dtype=W.dtype)
        for h in range(HPC):
            src = W[:, (h0 + h) * DH:(h0 + h) * DH + DH]
            base = h * ROT
            out[:, base:base + 16] = -src[:, 16:32]
            out[:, base + 16:base + 32] = src[:, 0:16]
        return out

    wqr = rot_cols(gW) * scale
    wkr = rot_cols(Wk_full)
    wo = np.concatenate([Wo[cols, :][m * P:(m + 1) * P, :] for m in range(2)],
                        axis=1)

    cosf = np.cos(pos_emb.T).astype(np.float32)
    sinf = np.sin(pos_emb.T).astype(np.float32)
    cos128 = np.ones((P, n), np.float32)
    cos128[0:ROT] = cosf
    cos128[DH:DH + ROT] = cosf
    sinc128 = np.empty((P, n), np.float32)
    for h in range(HPC):
        sinc128[h * ROT:(h + 1) * ROT] = sinf
    tri01 = (np.arange(P)[:, None] <= np.arange(P)[None, :]).astype(np.float32)

    ins = {
        "xT": np.ascontiguousarray(x[b].T).astype(ndt),
        "wq": _tmajor(wq).astype(ndt), "wk": _tmajor(wk).astype(ndt),
        "wv": _tmajor(wv).astype(ndt), "wqr": _tmajor(wqr).astype(ndt),
        "wkr": _tmajor(wkr).astype(ndt), "wo": wo.astype(ndt),
        "cos128": cos128, "sinc128": sinc128, "tri01": tri01.astype(ndt),
        "ident": np.eye(P, dtype=np.float32),
    }
    if not mask.all():
        km = np.where(mask[b], 0.0, NEG).astype(np.float32)
        ins["kmask"] = np.ascontiguousarray(km.reshape(n // P, P).T)
    return ins


# ---------------------------------------------------------------- runner

import os
import jax


def _run_per_device(nc, in_maps, core_ids):
    """Run the same Bass program independently on each visible device."""
    from concourse.bass2jax import (_bass_exec_p, install_neuronx_cc_hook,
                                    partition_id_tensor)
    install_neuronx_cc_hook()
    partition_name = nc.partition_id_tensor.name if nc.partition_id_tensor else None
    in_names, out_names, out_avals, zero_outs = [], [], [], []
    for alloc in nc.m.functions[0].allocations:
        if not isinstance(alloc, mybir.MemoryLocationSet):
            continue
        name = alloc.memorylocations[0].name
        if alloc.kind == "ExternalInput":
            if name != partition_name:
                in_names.append(name)
        elif alloc.kind == "ExternalOutput":
            out_names.append(name)
            shape = tuple(alloc.tensor_shape)
            dtype = mybir.dt.np(alloc.dtype)
            out_avals.append(jax.core.ShapedArray(shape, dtype))
            zero_outs.append(np.zeros(shape, dtype))
    n_params = len(in_names)
    all_in_names = list(in_names) + list(out_names)
    if partition_name is not None:
        all_in_names.append(partition_name)
    donate = tuple(range(n_params, n_params + len(out_names)))

    def _body(*args):
        operands = list(args)
        if partition_name is not None:
            operands.append(partition_id_tensor())
        outs = _bass_exec_p.bind(
            *operands, out_avals=tuple(out_avals), in_names=tuple(all_in_names),
            out_names=tuple(out_names), lowering_input_output_aliases=(),
            sim_require_finite=True, sim_require_nnan=True, nc=nc)
        return tuple(outs)

    fn = jax.jit(_body, donate_argnums=donate, keep_unused=True)
    futures = []
    for c, in_map in zip(core_ids, in_maps):
        dev = jax.devices()[c]
        args = [jax.device_put(np.asarray(in_map[nm]), dev) for nm in in_names]
        zz = [jax.device_put(z, dev) for z in zero_outs]
        futures.append(fn(*args, *zz))
    return [{nm: np.asarray(a) for nm, a in zip(out_names, f)} for f in futures]


_PROGRAM_CACHE = {}

MM_DT = "bf16"


def kernel(**inputs):
    os.environ.setdefault("NEURON_COMPILE_CACHE_URL", "/tmp/neuron_cache_kernel")
    x = np.asarray(inputs["x"], dtype=np.float32)
    mask = np.asarray(inputs["mask"]).astype(bool)
    pos_emb = np.asarray(inputs["pos_emb"], dtype=np.float32)
    g = np.asarray(inputs["g"], dtype=np.float32)
    Wq = np.asarray(inputs["Wq"], dtype=np.float32)
    Wkv = np.asarray(inputs["Wkv"], dtype=np.float32)
    Wo = np.asarray(inputs["Wo"], dtype=np.float32)
    bo = np.asarray(inputs["bo"], dtype=np.float32)
    b, n, _ = x.shape
    assert (b, n) == (2, 2048), (b, n)
    mm_dt = MM_DT
    use_km = not bool(mask.all())
    key = (n, mm_dt, use_km)
    if key not in _PROGRAM_CACHE:
        _PROGRAM_CACHE[key] = build_program(n=n, mm_dt=mm_dt, use_kmask=use_km)
    nc = _PROGRAM_CACHE[key]
    core_ids = list(range(8))
    in_maps = [make_core_inputs(x, mask, pos_emb, g, Wq, Wkv, Wo, c, n, mm_dt)
               for c in core_ids]
    results = _run_per_device(nc, in_maps, core_ids)
    out = np.zeros((b, n, D), np.float32)
    for c in core_ids:
        out[c // 4] += results[c]["out"].astype(np.float32)
    out += bo[None, None, :]
    return out
